# revision 26
# baseline (speedup 1.0000x reference)
"""CritiGraph update-step kernel for 8 Trainium2 NeuronCores (Bass/Tile).

Contract: kernel(**inputs) takes the FULL (unsharded) inputs of
reference.setup_inputs() and returns the FULL output
(np.stack([tl, pl, nl]), new_locations).

Strategy:
 - Data-parallel over batch: core m computes batch rows [16m, 16m+16).
 - Device layout: SBUF partitions = (b_local, t) = 16*8 = 128; free dims
   = (candidate, neighbor d). Both xor operands of the hot loop are
   rank-broadcast (stride-0) APs, so nothing big is ever materialized
   twice.
 - lut is the bit-length table (floor(log2(x))+1); computed on device
   arithmetically: int->f32 convert, exponent extract, Relu(e-126).
   Verified on host; falls back to a numpy reference if it mismatches.
 - Gathers of locations/degree rows are indirect DMAs from a host-packed
   locdeg [N, 12] array (cols 0-7 locations, col 8 degree) using wrapped
   indices; gathered in [64(d), (b,t)] layout, then one PE transpose per
   side -> [(b,t), d].
 - Launch B: each core copies its 12500-row locations shard DRAM->DRAM
   and indirect-scatters its assigned updated rows (disjoint per shard).
"""

import numpy as np

# ---- problem constants (hardcoded; kernel.py must be self-contained) ----
H, K, TP = 15, 15, 8
NLOC = 2 ** H                    # 32768
NNODES, BATCH, DMAX = 100000, 128, 64
NCAND = H * K + 1                # 226
EPS, GAMMA, ALPHA, POS_RATIO = 0.1, 2.0, 1.0, 1.0
NCORES = 8
NB = BATCH // NCORES             # 16 batch rows per core
SHARD = NNODES // NCORES         # 12500 location rows per core
P = 128
LDW = 12                         # locdeg row width (8 loc + 1 deg + 3 pad)
CCHUNKS = [29, 29, 29, 29, 29, 29, 29, 23]  # candidate chunks (sum = 226)
BIGF = 65536.0                   # tie-break sentinel (exact in f32)

_CACHE = {}
LAST_TRACE_DIR_A = None
LAST_TRACE_DIR_B = None
LAST_EXEC_NS = None
LAST_EXEC_NS_A = None
LAST_EXEC_NS_B = None


def _bitlen_table():
    xs = np.arange(NLOC)
    return np.where(xs == 0, 0.0,
                    np.floor(np.log2(np.maximum(xs, 1))) + 1.0).astype(np.float32)


# --------------------------------------------------------------------------
# device kernels
# --------------------------------------------------------------------------

def _reg_consts(nc, vals):
    import concourse.mybir as mybir
    for v in vals:
        t = nc.alloc_sbuf_tensor(f"const-float32-{v}", [128, 1], mybir.dt.float32)
        nc.gpsimd.memset(t.ap(), v)
        nc.const_aps.aps[(mybir.dt.float32, v)] = t.ap()


def _legalize_waits(nc, max_waits=1):
    """walrus CoreV3 codegen accepts only one sync-wait command per
    instruction; hoist extras onto preceding NoOps on the same engine."""
    import concourse.mybir as mybir
    n = 0
    for func in nc.m.functions:
        for bb in func.blocks:
            out = []
            for ins in bb.instructions:
                si = getattr(ins, "sync_info", None)
                waits = list(si.on_wait) if si is not None and si.on_wait else []
                if len(waits) > max_waits:
                    for w in waits[:-max_waits]:
                        out.append(mybir.InstNoOp(
                            name=f"{ins.name}-w{n}", engine=ins.engine,
                            ins=[], outs=[],
                            sync_info=mybir.SyncInfo(on_wait=[w], on_update=[])))
                        n += 1
                    si.on_wait = waits[-max_waits:]
                out.append(ins)
            bb.instructions = out
    return n


def _build_launch_A(fix8192):
    import concourse.bass as bass
    import concourse.mybir as mybir
    from concourse.tile import TileContext

    i32, f32 = mybir.dt.int32, mybir.dt.float32
    i16 = mybir.dt.int16
    A = mybir.AluOpType
    ACT = mybir.ActivationFunctionType

    nc = bass.Bass()
    _reg_consts(nc, [-126.0, 0.1, 1.0 + EPS])

    dp = nc.declare_dram_parameter
    locdeg_in = dp("locdeg", [NNODES, LDW], i32, isOutput=False)
    sta_bt_in = dp("sta_bt", [P, 1], i32, isOutput=False)       # repeat(sta,8)
    posT_in = dp("posT", [DMAX, NB], i32, isOutput=False)      # wrapped
    negT_in = dp("negT", [DMAX, NB], i32, isOutput=False)
    pos_bt_in = dp("pos_bt", [P, DMAX], i32, isOutput=False)    # raw (-1s)
    rmask_bt_in = dp("rmask_bt", [P, NCAND - 1], i32, isOutput=False)
    flip_in = dp("flip_rep", [P, NCAND - 1], i32, isOutput=False)
    pp_in = dp("pp_rep", [P, NCAND], f32, isOutput=False)
    ident_in = dp("ident64", [DMAX, DMAX], f32, isOutput=False)
    tsum_in = dp("tsum", [P, NB], f32, isOutput=False)
    trep_in = dp("trep", [NB, P], f32, isOutput=False)
    trepq_in = dp("trepq", [2 * NB, 2 * P], f32, isOutput=False)  # [q0|q1] stacked
    tsel_in = dp("tsel", [P, LDW], i32, isOutput=False)

    out_sel = dp("out_sel", [P, 8], f32, isOutput=True)

    with TileContext(nc) as tc:
        with (
            tc.tile_pool(name="persist", bufs=1) as pp_pool,
            tc.tile_pool(name="big", bufs=3) as big_pool,
            tc.tile_pool(name="red", bufs=16) as red_pool,
            tc.tile_pool(name="psum", bufs=4, space="PSUM") as ps_pool,
        ):
            # ---- load small inputs ----
            def load(name, src, shape, dtype):
                t = pp_pool.tile(shape, dtype, tag=name)
                nc.sync.dma_start(out=t[:], in_=src[:])
                return t

            sta_bt = load("sta_bt", sta_bt_in, [P, 1], i32)
            posT = load("posT", posT_in, [DMAX, NB], i32)
            negT = load("negT", negT_in, [DMAX, NB], i32)
            pos_bt = load("pos_bt", pos_bt_in, [P, DMAX], i32)
            rmask_bt = load("rmask_bt", rmask_bt_in, [P, NCAND - 1], i32)
            flip_rep = load("flip_rep", flip_in, [P, NCAND - 1], i32)
            pp_rep = load("pp_rep", pp_in, [P, NCAND], f32)
            ident64 = load("ident64", ident_in, [DMAX, DMAX], f32)
            tsum = load("tsum", tsum_in, [P, NB], f32)
            trep = load("trep", trep_in, [NB, P], f32)
            trepq = load("trepq", trepq_in, [2 * NB, 2 * P], f32)
            tsel = load("tsel", tsel_in, [P, LDW], i32)

            # ---- gathers ----
            # sta row: locdeg[sta] -> [128, 12]
            sta_row = pp_pool.tile([P, LDW], i32)
            nc.gpsimd.indirect_dma_start(
                out=sta_row[:], out_offset=None, in_=locdeg_in[:],
                in_offset=bass.IndirectOffsetOnAxis(ap=sta_bt[:, :1], axis=0))
            # sta_loc[p] = sta_row[p, p%8] via one-hot tsel
            stp = pp_pool.tile([P, LDW], i32)
            nc.vector.tensor_tensor(out=stp[:], in0=sta_row[:], in1=tsel[:], op=A.mult)
            sta_loc = pp_pool.tile([P, 1], i32)
            with nc.allow_low_precision(reason="int32 one-hot reduce is exact"):
                nc.vector.tensor_reduce(out=sta_loc[:], in_=stp[:],
                                        axis=mybir.AxisListType.X, op=A.add)
            deg1f = pp_pool.tile([P, 1], f32)
            nc.vector.tensor_copy(out=deg1f[:], in_=sta_row[:, 8:9])
            deg1p1 = pp_pool.tile([P, 1], f32)
            nc.vector.tensor_scalar(out=deg1p1[:], in0=deg1f[:], scalar1=1.0,
                                    scalar2=None, op0=A.add)
            invlg = pp_pool.tile([P, 1], f32)
            nc.vector.reciprocal(out=invlg[:], in_=deg1f[:])

            # neighbor rows, transposed-gather per b: [64(d), 16(b), 12]
            rows = {}
            for side, idxT in (("p", posT), ("n", negT)):
                rt = pp_pool.tile([DMAX, NB, LDW], i32, tag=f"rows_{side}")
                for b in range(NB):
                    nc.gpsimd.indirect_dma_start(
                        out=rt[:, b, :], out_offset=None, in_=locdeg_in[:],
                        in_offset=bass.IndirectOffsetOnAxis(ap=idxT[:, b:b + 1], axis=0))
                rows[side] = rt

            # ---- PE transposes to (b,t) layout ----
            loc_bt = {}
            for side in ("p", "n"):
                lf = pp_pool.tile([DMAX, P], f32, tag=f"locf_{side}")
                nc.vector.tensor_copy(
                    out=lf[:].rearrange("d (b t) -> d b t", b=NB),
                    in_=rows[side][:, :, 0:8])
                ps = ps_pool.tile([P, DMAX], f32, tag="ps_small")
                nc.tensor.transpose(out=ps[:], in_=lf[:], identity=ident64[:])
                li = pp_pool.tile([P, DMAX], i32, tag=f"loc_bt_{side}")
                nc.vector.tensor_copy(out=li[:], in_=ps[:])
                loc_bt[side] = li

            # deg2 columns stacked [64, 32] f32 -> transpose -> [32, 64]
            degs = pp_pool.tile([DMAX, 2 * NB], f32)
            nc.vector.tensor_copy(out=degs[:, 0:NB], in_=rows["p"][:, :, 8])
            nc.vector.tensor_copy(out=degs[:, NB:2 * NB], in_=rows["n"][:, :, 8])
            degsT_ps = ps_pool.tile([2 * NB, DMAX], f32, tag="ps_small")
            nc.tensor.transpose(out=degsT_ps[:], in_=degs[:], identity=ident64[:])
            degsT = pp_pool.tile([2 * NB, DMAX], f32)
            nc.vector.tensor_copy(out=degsT[:], in_=degsT_ps[:])

            # replicate b -> (b,t): deg2 side reps [128, 64]
            deg2_rep = {}
            for qi, side in ((0, "p"), (1, "n")):
                psd = ps_pool.tile([P, DMAX], f32, tag="ps_small")
                nc.tensor.matmul(out=psd[:], lhsT=trepq[:, qi * P:(qi + 1) * P],
                                 rhs=degsT[:], start=True, stop=True)
                deg2_rep[side] = psd

            # ---- mask, iv, U, W per side ----
            mask_f = pp_pool.tile([P, DMAX], f32)
            nc.vector.tensor_scalar(out=mask_f[:], in0=pos_bt[:], scalar1=0,
                                    scalar2=None, op0=A.is_ge)

            U, W = {}, {}
            for side in ("p", "n"):
                # X = sta ^ loc ; S = bitlen(X)
                X = pp_pool.tile([P, DMAX], i32, tag=f"X_{side}")
                nc.vector.tensor_tensor(out=X[:], in0=loc_bt[side][:],
                                        in1=sta_loc[:].to_broadcast([P, DMAX]),
                                        op=A.bitwise_xor)
                Xf = pp_pool.tile([P, DMAX], f32, tag=f"Xf_{side}")
                nc.scalar.copy(out=Xf[:], in_=X[:])
                e_t = pp_pool.tile([P, DMAX], i32, tag=f"e_{side}")
                nc.vector.tensor_scalar(out=e_t[:], in0=Xf[:].bitcast(i32),
                                        scalar1=23, scalar2=None,
                                        op0=A.logical_shift_right)
                srelu_scale = 1.0
                if fix8192:
                    # reference lut[8192] is 13 (f32 log2 artifact), not 14
                    e_t2 = pp_pool.tile([P, DMAX], i32, tag=f"e2_{side}")
                    nc.vector.scalar_tensor_tensor(
                        out=e_t2[:], in0=Xf[:], scalar=8192.0, in1=e_t[:],
                        op0=A.is_equal, op1=A.subtract)
                    e_t = e_t2
                    srelu_scale = -1.0
                S = pp_pool.tile([P, DMAX], f32, tag=f"S_{side}")
                nc.scalar.activation(out=S[:], in_=e_t[:], func=ACT.Relu,
                                     bias=-126.0, scale=srelu_scale)
                # sumS over t (within b), then replicate back
                s16 = ps_pool.tile([NB, DMAX], f32, tag="ps_small")
                nc.tensor.matmul(out=s16[:], lhsT=tsum[:], rhs=S[:],
                                 start=True, stop=True)
                s16s = pp_pool.tile([NB, DMAX], f32, tag=f"s16_{side}")
                nc.vector.tensor_copy(out=s16s[:], in_=s16[:])
                srep = ps_pool.tile([P, DMAX], f32, tag="ps_small")
                nc.tensor.matmul(out=srep[:], lhsT=trep[:], rhs=s16s[:],
                                 start=True, stop=True)
                A_t = pp_pool.tile([P, DMAX], f32, tag=f"A_{side}")
                nc.vector.tensor_tensor(out=A_t[:], in0=srep[:], in1=S[:],
                                        op=A.subtract)
                # iv = 1/ln((deg1+1)(deg2+1))
                lp = pp_pool.tile([P, DMAX], f32, tag=f"lp_{side}")
                nc.vector.tensor_scalar(out=lp[:], in0=deg2_rep[side][:],
                                        scalar1=1.0, scalar2=deg1p1[:, :1],
                                        op0=A.add, op1=A.mult)
                lga = pp_pool.tile([P, DMAX], f32, tag=f"lga_{side}")
                nc.scalar.activation(out=lga[:], in_=lp[:], func=ACT.Ln,
                                     bias=0.0, scale=1.0)
                iv = pp_pool.tile([P, DMAX], f32, tag=f"iv_{side}")
                nc.vector.reciprocal(out=iv[:], in_=lga[:])
                # U = mask*iv/8
                Ut = pp_pool.tile([P, DMAX], f32, tag=f"U_{side}")
                nc.vector.scalar_tensor_tensor(out=Ut[:], in0=mask_f[:],
                                               scalar=0.125, in1=iv[:],
                                               op0=A.mult, op1=A.mult)
                U[side] = Ut
                # W
                Wt = pp_pool.tile([P, DMAX], f32, tag=f"W_{side}")
                if side == "p":
                    t1 = pp_pool.tile([P, DMAX], f32, tag="w_t1_p")
                    nc.vector.scalar_tensor_tensor(out=t1[:], in0=A_t[:],
                                                   scalar=0.125, in1=mask_f[:],
                                                   op0=A.mult, op1=A.mult)
                    nc.vector.scalar_tensor_tensor(out=Wt[:], in0=t1[:],
                                                   scalar=EPS, in1=iv[:],
                                                   op0=A.add, op1=A.mult)
                else:
                    t1 = pp_pool.tile([P, DMAX], f32, tag="w_t1_n")
                    nc.vector.tensor_scalar(out=t1[:], in0=A_t[:], scalar1=0.125,
                                            scalar2=1000.0, op0=A.mult,
                                            op1=A.subtract)
                    t2 = pp_pool.tile([P, DMAX], f32, tag="w_t2_n")
                    nc.vector.tensor_tensor(out=t2[:], in0=t1[:], in1=mask_f[:],
                                            op=A.mult)
                    nc.vector.scalar_tensor_tensor(out=Wt[:], in0=t2[:],
                                                   scalar=1000.0 + EPS, in1=iv[:],
                                                   op0=A.add, op1=A.mult)
                W[side] = Wt

            # ---- candidate values cncv [128, 226] ----
            cncv = pp_pool.tile([P, NCAND], i32)
            nc.vector.tensor_tensor(out=cncv[:, 0:NCAND - 1], in0=rmask_bt[:],
                                    in1=flip_rep[:], op=A.bitwise_xor)
            nc.vector.memset(cncv[:, NCAND - 1:NCAND], 0)
            nc.vector.tensor_tensor(out=cncv[:], in0=cncv[:],
                                    in1=sta_loc[:].to_broadcast([P, NCAND]),
                                    op=A.bitwise_xor)
            cncf = pp_pool.tile([P, NCAND], f32)
            nc.vector.tensor_copy(out=cncf[:], in_=cncv[:])

            # ---- big loop ----
            posred = pp_pool.tile([P, NCAND], f32)
            negred = pp_pool.tile([P, NCAND], f32)
            red_parts = []
            c0 = 0
            for cc in CCHUNKS:
                cs = slice(c0, c0 + cc)
                cv_b = cncv[:, cs].unsqueeze(2).to_broadcast([P, cc, DMAX])
                for side, red in (("p", posred), ("n", negred)):
                    lb_b = loc_bt[side][:].unsqueeze(1).to_broadcast([P, cc, DMAX])
                    U_b = U[side][:].unsqueeze(1).to_broadcast([P, cc, DMAX])
                    W_b = W[side][:].unsqueeze(1).to_broadcast([P, cc, DMAX])

                    y = big_pool.tile([P, cc, DMAX], i32, tag="tA")
                    nc.vector.tensor_tensor(out=y[:], in0=cv_b, in1=lb_b,
                                            op=A.bitwise_xor)
                    yf = big_pool.tile([P, cc, DMAX], f32, tag="tB")
                    nc.scalar.copy(out=yf[:], in_=y[:])
                    e_b = big_pool.tile([P, cc, DMAX], i32, tag="tC")
                    nc.vector.tensor_scalar(out=e_b[:], in0=yf[:].bitcast(i32),
                                            scalar1=23, scalar2=None,
                                            op0=A.logical_shift_right)
                    relu_scale = 1.0
                    if fix8192:
                        e_b2 = big_pool.tile([P, cc, DMAX], i32, tag="tD")
                        nc.vector.scalar_tensor_tensor(
                            out=e_b2[:], in0=yf[:], scalar=8192.0, in1=e_b[:],
                            op0=A.is_equal, op1=A.subtract)
                        e_b = e_b2
                        relu_scale = -1.0
                    Lf = big_pool.tile([P, cc, DMAX], f32, tag="tE")
                    nc.scalar.activation(out=Lf[:], in_=e_b[:], func=ACT.Relu,
                                         bias=-126.0, scale=relu_scale)
                    t1 = big_pool.tile([P, cc, DMAX], f32, tag="tF")
                    nc.vector.tensor_tensor(out=t1[:], in0=Lf[:], in1=U_b,
                                            op=A.mult)
                    aa = big_pool.tile([P, cc, DMAX], f32, tag="tG")
                    nc.vector.tensor_tensor(out=aa[:], in0=t1[:], in1=W_b,
                                            op=A.add)
                    q = big_pool.tile([P, cc, DMAX], f32, tag="tA")
                    nc.scalar.activation(out=q[:], in_=aa[:], func=ACT.Square)
                    lnp = big_pool.tile([P, cc, DMAX], f32, tag="tB")
                    nc.scalar.activation(out=lnp[:], in_=q[:], func=ACT.Ln,
                                         bias=1.0, scale=1.0)
                    if side == "p":
                        r = lnp
                    else:
                        ln2 = big_pool.tile([P, cc, DMAX], f32, tag="tC")
                        nc.scalar.activation(out=ln2[:], in_=q[:], func=ACT.Ln,
                                             bias=0.1, scale=1.0 + EPS)
                        r = big_pool.tile([P, cc, DMAX], f32, tag="tD")
                        nc.vector.tensor_tensor(out=r[:], in0=lnp[:], in1=ln2[:],
                                                op=A.subtract)
                    rc = red_pool.tile([P, cc], f32, tag="tred")
                    nc.vector.tensor_reduce(out=rc[:], in_=r[:],
                                            axis=mybir.AxisListType.X, op=A.add)
                    red_parts.append((red, cs, rc))
                c0 += cc
            for red, cs, rc in red_parts:
                nc.vector.tensor_copy(out=red[:, cs], in_=rc[:])

            # ---- losses, tie-aware argmin, selection ----
            tsum2 = pp_pool.tile([P, NCAND], f32)
            nc.vector.tensor_tensor(out=tsum2[:], in0=posred[:], in1=negred[:],
                                    op=A.add)
            total = pp_pool.tile([P, NCAND], f32)
            nc.vector.tensor_scalar(out=total[:], in0=tsum2[:],
                                    scalar1=invlg[:, :1], scalar2=None, op0=A.mult)
            minv = pp_pool.tile([P, 1], f32)
            nc.vector.tensor_reduce(out=minv[:], in_=total[:],
                                    axis=mybir.AxisListType.X, op=A.min)
            eqm = pp_pool.tile([P, NCAND], i32)
            nc.vector.tensor_scalar(out=eqm[:], in0=total[:], scalar1=minv[:, :1],
                                    scalar2=None, op0=A.is_equal)
            bigc = pp_pool.tile([P, 1], f32)
            nc.vector.memset(bigc[:], BIGF)
            score = pp_pool.tile([P, NCAND], f32)
            nc.vector.select(out=score[:], mask=eqm[:], on_true=pp_rep[:],
                             on_false=bigc[:].to_broadcast([P, NCAND]))
            pstar = pp_pool.tile([P, 1], f32)
            nc.vector.tensor_reduce(out=pstar[:], in_=score[:],
                                    axis=mybir.AxisListType.X, op=A.min)
            mask2 = pp_pool.tile([P, NCAND], f32)
            nc.vector.tensor_scalar(out=mask2[:], in0=score[:], scalar1=pstar[:, :1],
                                    scalar2=None, op0=A.is_equal)

            sel = pp_pool.tile([P, 8], f32)
            nc.vector.memset(sel[:], 0.0)
            scrap = pp_pool.tile([P, NCAND], f32)
            for col, val in ((0, posred), (1, negred), (3, cncf)):
                nc.vector.scalar_tensor_tensor(
                    out=scrap[:], in0=mask2[:], scalar=1.0, in1=val[:],
                    op0=A.mult, op1=A.mult, accum_out=sel[:, col:col + 1])
            nc.sync.dma_start(out=out_sel[:], in_=sel[:])

    _legalize_waits(nc)
    return nc


def _build_launch_B():
    import concourse.bass as bass
    import concourse.mybir as mybir
    from concourse.tile import TileContext

    i32 = mybir.dt.int32
    nc = bass.Bass()
    dp = nc.declare_dram_parameter
    shard_in = dp("shard_in", [SHARD, TP], i32, isOutput=False)
    scat_idx_in = dp("scat_idx", [P, 1], i32, isOutput=False)
    scat_val_in = dp("scat_val", [P, TP], i32, isOutput=False)
    shard_out = dp("shard_out", [SHARD, TP], i32, isOutput=True)

    with TileContext(nc) as tc:
        with tc.tile_pool(name="sbuf", bufs=1) as pool:
            nc.sync.dma_start(out=shard_out[:], in_=shard_in[:])
            sidx = pool.tile([P, 1], i32)
            nc.sync.dma_start(out=sidx[:], in_=scat_idx_in[:])
            sval = pool.tile([P, TP], i32)
            nc.sync.dma_start(out=sval[:], in_=scat_val_in[:])
            nc.gpsimd.indirect_dma_start(
                out=shard_out[:],
                out_offset=bass.IndirectOffsetOnAxis(ap=sidx[:, :1], axis=0),
                in_=sval[:], in_offset=None,
                bounds_check=SHARD - 1, oob_is_err=False)
    _legalize_waits(nc)
    return nc


# --------------------------------------------------------------------------
# host-side driver
# --------------------------------------------------------------------------

def _consts():
    flip_jk = np.repeat((1 << np.arange(H)).astype(np.int32), K)       # [225]
    tsum = np.zeros((P, NB), np.float32)
    tsum[np.arange(P), np.arange(P) // TP] = 1.0
    trep = np.zeros((NB, P), np.float32)
    trep[np.arange(P) // TP, np.arange(P)] = 1.0
    trepq = np.zeros((2 * NB, 2 * P), np.float32)
    trepq[np.arange(P) // TP, np.arange(P)] = 1.0                 # q0 block
    trepq[NB + np.arange(P) // TP, P + np.arange(P)] = 1.0        # q1 block
    tsel = np.zeros((P, LDW), np.int32)
    tsel[np.arange(P), np.arange(P) % TP] = 1
    ident64 = np.eye(DMAX, dtype=np.float32)
    return flip_jk, tsum, trep, trepq, tsel, ident64


def _wrap(idx):
    return np.where(idx < 0, idx + NNODES, idx).astype(np.int32)


def _numpy_ref(lut, sta_ind, locations, degree, pos_ind, neg_ind, random_masks, perm):
    """Exact numpy fallback mirroring reference.py (used only if lut is not
    the bit-length table)."""
    mask = pos_ind != -1
    lg = mask.sum(axis=1).astype(np.float32)
    sta_loc = locations[sta_ind]
    pos_loc = np.where(mask[:, :, None], locations[np.where(mask, pos_ind, 0)], -1)
    neg_loc = np.where(mask[:, :, None], locations[np.where(mask, neg_ind, 0)], -1)
    flip = (1 << np.arange(H, dtype=np.int32))[None, :, None]
    flipped = sta_loc[:, None, :] ^ flip
    cand = (flipped[:, :, None, :] ^ random_masks).reshape(BATCH, H * K, TP)
    cnc_loc = np.concatenate([cand, sta_loc[:, None, :]], axis=1)[:, perm, :]
    m1 = mask[:, :, None]
    m2 = mask[:, :, None, None]

    def dist(xor, m):
        return np.where(m, lut[np.where(m, xor, 0)], -1.0).astype(np.float32)

    dsp = dist(sta_loc[:, None, :] ^ pos_loc, m1)
    dsps = dsp.sum(axis=-1)
    dsn = dist(sta_loc[:, None, :] ^ neg_loc, m1)
    dsns = dsn.sum(axis=-1)
    dpc = dist(cnc_loc[:, None, :, :] ^ pos_loc[:, :, None, :], m2)
    dnc = dist(cnc_loc[:, None, :, :] ^ neg_loc[:, :, None, :], m2)
    dnp_ = np.where(m2, (dpc - dsp[:, :, None, :] + dsps[:, :, None, None]) / TP, 0.0)
    dnn = np.where(m2, (dnc - dsn[:, :, None, :] + dsns[:, :, None, None]) / TP, 1000.0)

    def p_(dis, ig2):
        deg1 = degree[sta_ind][:, None]
        ig2w = np.where(ig2 < 0, ig2 + NNODES, ig2)
        deg2 = degree[ig2w]
        log_ap = np.log(((deg1 + 1) * (deg2 + 1)).astype(np.float32))
        aaq = (dis + EPS) / log_ap[:, :, None, None]
        return 1.0 / (1.0 + aaq ** GAMMA / ALPHA)

    pos_loss = -np.sum(np.log(p_(dnp_, pos_ind)), axis=1) / lg[:, None, None]
    neg_loss = -np.sum(np.log(1.0 + EPS - p_(dnn, neg_ind)), axis=1) / lg[:, None, None]
    total = POS_RATIO * pos_loss + neg_loss
    index = np.argmin(total, axis=1)
    sel_loc = np.take_along_axis(cnc_loc, index[:, None, :], axis=1)[:, 0, :]
    new_loc = locations.copy()
    new_loc[sta_ind] = sel_loc

    def gsel(x):
        return np.take_along_axis(x, index[:, None, :], axis=1)[:, 0, :]

    return (np.stack([gsel(total).mean(), gsel(pos_loss).mean(),
                      gsel(neg_loss).mean()]).astype(np.float32),
            new_loc.astype(np.int32))


def _ensure_ntff_hook():
    """Register the NTFF profiling hook this image's antenv lacks, and stub
    out the artifact upload (no bucket access here)."""
    import sys, types
    if "antenv.axon_hooks" not in sys.modules:
        mod = types.ModuleType("antenv.axon_hooks")
        holder = {}
        mod.set_axon_ntff_profile_hook = lambda h: holder.__setitem__("h", h)
        mod.get_axon_ntff_profile_hook = lambda: holder.get("h")
        try:
            from trn_agent_boot.trn_boot import _ntff_profile_via_ctypes
            hook = _ntff_profile_via_ctypes("/opt/axon/libaxon_pjrt.so")
            if hook is not None:
                holder["h"] = hook
        except Exception:
            pass
        sys.modules["antenv.axon_hooks"] = mod
        import antenv
        antenv.axon_hooks = mod
    import concourse.bass_utils as bu
    bu.upload_artifacts = lambda tmpdir: f"local:{tmpdir}"


def kernel(lut, sta_ind, locations, degree, pos_ind, neg_ind, random_masks, perm):
    lut = np.asarray(lut, np.float32)
    sta_ind = np.asarray(sta_ind, np.int32)
    locations = np.asarray(locations, np.int32)
    degree = np.asarray(degree, np.int32)
    pos_ind = np.asarray(pos_ind, np.int32)
    neg_ind = np.asarray(neg_ind, np.int32)
    random_masks = np.asarray(random_masks, np.int32)
    perm = np.asarray(perm, np.int32)

    tab = _bitlen_table()
    if np.array_equal(lut, tab):
        fix8192 = False
    else:
        d = tab - lut
        only8192 = (d[NLOC // 4] == 1.0) and (np.count_nonzero(d) == 1)
        if only8192:
            fix8192 = True
        else:
            return _numpy_ref(lut, sta_ind, locations, degree, pos_ind, neg_ind,
                              random_masks, perm)

    from concourse.bass_utils import run_bass_kernel_spmd

    key = ("A", fix8192)
    if key not in _CACHE:
        _CACHE[key] = _build_launch_A(fix8192)
        _CACHE["B"] = _build_launch_B()
    _CACHE["A"] = _CACHE[key]

    flip_jk, tsum, trep, trepq, tsel, ident64 = _consts()
    pp = np.empty(NCAND, np.float32)
    pp[perm] = np.arange(NCAND, dtype=np.float32)
    pp_rep = np.tile(pp[None, :], (P, 1))
    flip_rep = np.tile(flip_jk[None, :], (P, 1)).astype(np.int32)

    locdeg = np.zeros((NNODES, LDW), np.int32)
    locdeg[:, 0:TP] = locations
    locdeg[:, 8] = degree

    in_maps_A = []
    for m in range(NCORES):
        bs = slice(m * NB, (m + 1) * NB)
        pos_l, neg_l = pos_ind[bs], neg_ind[bs]
        in_maps_A.append(dict(
            locdeg=locdeg,
            sta_bt=np.repeat(sta_ind[bs], TP).reshape(P, 1).astype(np.int32),
            posT=np.ascontiguousarray(_wrap(pos_l).T),
            negT=np.ascontiguousarray(_wrap(neg_l).T),
            pos_bt=np.repeat(pos_l, TP, axis=0).astype(np.int32),
            rmask_bt=np.ascontiguousarray(
                random_masks[bs].transpose(0, 3, 1, 2).reshape(P, H * K)).astype(np.int32),
            flip_rep=flip_rep, pp_rep=pp_rep, ident64=ident64,
            tsum=tsum, trep=trep, trepq=trepq, tsel=tsel,
        ))
    import os, tempfile
    trace = bool(os.environ.get("KERNEL_TRACE"))
    tkw = {}
    global LAST_TRACE_DIR_A, LAST_TRACE_DIR_B
    if trace:
        _ensure_ntff_hook()
        LAST_TRACE_DIR_A = tempfile.mkdtemp(prefix="ktrA_")
        tkw = dict(trace=True, tmpdir=LAST_TRACE_DIR_A)
    resA = run_bass_kernel_spmd(_CACHE["A"], in_maps_A, list(range(NCORES)), **tkw)
    global LAST_EXEC_NS_A
    LAST_EXEC_NS_A = resA.exec_time_ns
    sel = np.stack([resA.results[m]["out_sel"] for m in range(NCORES)])  # [8,128,8]

    sel_bt = sel.reshape(NCORES, NB, TP, 8).reshape(BATCH, TP, 8)
    lg = (pos_ind != -1).sum(axis=1).astype(np.float32)                  # [128]
    pos_sel = sel_bt[:, :, 0] / lg[:, None]
    neg_sel = sel_bt[:, :, 1] / lg[:, None]
    tl = float((pos_sel + neg_sel).mean())
    pl = float(pos_sel.mean())
    nl = float(neg_sel.mean())
    sel_loc = np.rint(sel_bt[:, :, 3]).astype(np.int32)                  # [128, 8]

    # launch B: shard copy + disjoint scatter
    in_maps_B = []
    shard_of = sta_ind // SHARD
    for m in range(NCORES):
        sidx = np.full((P, 1), 1 << 20, np.int32)
        sval = np.zeros((P, TP), np.int32)
        rows = np.where(shard_of == m)[0]
        sidx[:len(rows), 0] = sta_ind[rows] - m * SHARD
        sval[:len(rows)] = sel_loc[rows]
        in_maps_B.append(dict(
            shard_in=np.ascontiguousarray(locations[m * SHARD:(m + 1) * SHARD]),
            scat_idx=sidx, scat_val=sval,
        ))
    tkwB = {}
    if trace:
        LAST_TRACE_DIR_B = tempfile.mkdtemp(prefix="ktrB_")
        tkwB = dict(trace=True, tmpdir=LAST_TRACE_DIR_B)
    resB = run_bass_kernel_spmd(_CACHE["B"], in_maps_B, list(range(NCORES)), **tkwB)
    global LAST_EXEC_NS_B, LAST_EXEC_NS
    LAST_EXEC_NS_B = resB.exec_time_ns
    if LAST_EXEC_NS_A is not None and LAST_EXEC_NS_B is not None:
        LAST_EXEC_NS = LAST_EXEC_NS_A + LAST_EXEC_NS_B
    new_loc = np.concatenate(
        [resB.results[m]["shard_out"] for m in range(NCORES)], axis=0)

    return (np.stack([tl, pl, nl]).astype(np.float32), new_loc.astype(np.int32))


# revision 27
# speedup vs baseline: 1.0202x; 1.0202x over previous
"""CritiGraph update-step kernel for 8 Trainium2 NeuronCores (Bass/Tile).

Contract: kernel(**inputs) takes the FULL (unsharded) inputs of
reference.setup_inputs() and returns the FULL output
(np.stack([tl, pl, nl]), new_locations).

Strategy:
 - Data-parallel over batch: core m computes batch rows [16m, 16m+16).
 - Device layout: SBUF partitions = (b_local, t) = 16*8 = 128; free dims
   = (candidate, neighbor d). Both xor operands of the hot loop are
   rank-broadcast (stride-0) APs, so nothing big is ever materialized
   twice.
 - lut is the bit-length table (floor(log2(x))+1); computed on device
   arithmetically: int->f32 convert, exponent extract, Relu(e-126).
   Verified on host; falls back to a numpy reference if it mismatches.
 - Gathers of locations/degree rows are indirect DMAs from a host-packed
   locdeg [N, 12] array (cols 0-7 locations, col 8 degree) using wrapped
   indices; gathered in [64(d), (b,t)] layout, then one PE transpose per
   side -> [(b,t), d].
 - Launch B: each core copies its 12500-row locations shard DRAM->DRAM
   and indirect-scatters its assigned updated rows (disjoint per shard).
"""

import numpy as np

# ---- problem constants (hardcoded; kernel.py must be self-contained) ----
H, K, TP = 15, 15, 8
NLOC = 2 ** H                    # 32768
NNODES, BATCH, DMAX = 100000, 128, 64
NCAND = H * K + 1                # 226
EPS, GAMMA, ALPHA, POS_RATIO = 0.1, 2.0, 1.0, 1.0
NCORES = 8
NB = BATCH // NCORES             # 16 batch rows per core
SHARD = NNODES // NCORES         # 12500 location rows per core
P = 128
LDW = 12                         # locdeg row width (8 loc + 1 deg + 3 pad)
CCHUNKS = [38, 38, 38, 38, 37, 37]  # candidate chunks (sum = 226)
BIGF = 65536.0                   # tie-break sentinel (exact in f32)

_CACHE = {}
LAST_TRACE_DIR_A = None
LAST_TRACE_DIR_B = None
LAST_EXEC_NS = None
LAST_EXEC_NS_A = None
LAST_EXEC_NS_B = None


def _bitlen_table():
    xs = np.arange(NLOC)
    return np.where(xs == 0, 0.0,
                    np.floor(np.log2(np.maximum(xs, 1))) + 1.0).astype(np.float32)


# --------------------------------------------------------------------------
# device kernels
# --------------------------------------------------------------------------

def _reg_consts(nc, vals):
    import concourse.mybir as mybir
    for v in vals:
        t = nc.alloc_sbuf_tensor(f"const-float32-{v}", [128, 1], mybir.dt.float32)
        nc.gpsimd.memset(t.ap(), v)
        nc.const_aps.aps[(mybir.dt.float32, v)] = t.ap()


def _legalize_waits(nc, max_waits=1):
    """walrus CoreV3 codegen accepts only one sync-wait command per
    instruction; hoist extras onto preceding NoOps on the same engine."""
    import concourse.mybir as mybir
    n = 0
    for func in nc.m.functions:
        for bb in func.blocks:
            out = []
            for ins in bb.instructions:
                si = getattr(ins, "sync_info", None)
                waits = list(si.on_wait) if si is not None and si.on_wait else []
                if len(waits) > max_waits:
                    for w in waits[:-max_waits]:
                        out.append(mybir.InstNoOp(
                            name=f"{ins.name}-w{n}", engine=ins.engine,
                            ins=[], outs=[],
                            sync_info=mybir.SyncInfo(on_wait=[w], on_update=[])))
                        n += 1
                    si.on_wait = waits[-max_waits:]
                out.append(ins)
            bb.instructions = out
    return n


def _build_launch_A(fix8192):
    import concourse.bass as bass
    import concourse.mybir as mybir
    from concourse.tile import TileContext

    i32, f32 = mybir.dt.int32, mybir.dt.float32
    i16 = mybir.dt.int16
    A = mybir.AluOpType
    ACT = mybir.ActivationFunctionType

    nc = bass.Bass()
    _reg_consts(nc, [-126.0, 0.1, 1.0 + EPS])

    dp = nc.declare_dram_parameter
    locdeg_in = dp("locdeg", [NNODES, LDW], i32, isOutput=False)
    sta_bt_in = dp("sta_bt", [P, 1], i32, isOutput=False)       # repeat(sta,8)
    posT_in = dp("posT", [DMAX, NB], i32, isOutput=False)      # wrapped
    negT_in = dp("negT", [DMAX, NB], i32, isOutput=False)
    pos_bt_in = dp("pos_bt", [P, DMAX], i32, isOutput=False)    # raw (-1s)
    rmask_bt_in = dp("rmask_bt", [P, NCAND - 1], i32, isOutput=False)
    flip_in = dp("flip_rep", [P, NCAND - 1], i32, isOutput=False)
    pp_in = dp("pp_rep", [P, NCAND], f32, isOutput=False)
    ident_in = dp("ident64", [DMAX, DMAX], f32, isOutput=False)
    tsum_in = dp("tsum", [P, NB], f32, isOutput=False)
    trep_in = dp("trep", [NB, P], f32, isOutput=False)
    trepq_in = dp("trepq", [2 * NB, 2 * P], f32, isOutput=False)  # [q0|q1] stacked
    tsel_in = dp("tsel", [P, LDW], i32, isOutput=False)

    out_sel = dp("out_sel", [P, 8], f32, isOutput=True)

    with TileContext(nc) as tc:
        with (
            tc.tile_pool(name="persist", bufs=1) as pp_pool,
            tc.tile_pool(name="big", bufs=2) as big_pool,
            tc.tile_pool(name="red", bufs=16) as red_pool,
            tc.tile_pool(name="psum", bufs=4, space="PSUM") as ps_pool,
        ):
            # ---- load small inputs ----
            def load(name, src, shape, dtype):
                t = pp_pool.tile(shape, dtype, tag=name)
                nc.sync.dma_start(out=t[:], in_=src[:])
                return t

            sta_bt = load("sta_bt", sta_bt_in, [P, 1], i32)
            posT = load("posT", posT_in, [DMAX, NB], i32)
            negT = load("negT", negT_in, [DMAX, NB], i32)
            pos_bt = load("pos_bt", pos_bt_in, [P, DMAX], i32)
            rmask_bt = load("rmask_bt", rmask_bt_in, [P, NCAND - 1], i32)
            flip_rep = load("flip_rep", flip_in, [P, NCAND - 1], i32)
            pp_rep = load("pp_rep", pp_in, [P, NCAND], f32)
            ident64 = load("ident64", ident_in, [DMAX, DMAX], f32)
            tsum = load("tsum", tsum_in, [P, NB], f32)
            trep = load("trep", trep_in, [NB, P], f32)
            trepq = load("trepq", trepq_in, [2 * NB, 2 * P], f32)
            tsel = load("tsel", tsel_in, [P, LDW], i32)

            # ---- gathers ----
            # sta row: locdeg[sta] -> [128, 12]
            sta_row = pp_pool.tile([P, LDW], i32)
            nc.gpsimd.indirect_dma_start(
                out=sta_row[:], out_offset=None, in_=locdeg_in[:],
                in_offset=bass.IndirectOffsetOnAxis(ap=sta_bt[:, :1], axis=0))
            # sta_loc[p] = sta_row[p, p%8] via one-hot tsel
            stp = pp_pool.tile([P, LDW], i32)
            nc.vector.tensor_tensor(out=stp[:], in0=sta_row[:], in1=tsel[:], op=A.mult)
            sta_loc = pp_pool.tile([P, 1], i32)
            with nc.allow_low_precision(reason="int32 one-hot reduce is exact"):
                nc.vector.tensor_reduce(out=sta_loc[:], in_=stp[:],
                                        axis=mybir.AxisListType.X, op=A.add)
            deg1f = pp_pool.tile([P, 1], f32)
            nc.vector.tensor_copy(out=deg1f[:], in_=sta_row[:, 8:9])
            deg1p1 = pp_pool.tile([P, 1], f32)
            nc.vector.tensor_scalar(out=deg1p1[:], in0=deg1f[:], scalar1=1.0,
                                    scalar2=None, op0=A.add)
            invlg = pp_pool.tile([P, 1], f32)
            nc.vector.reciprocal(out=invlg[:], in_=deg1f[:])

            # neighbor rows, transposed-gather per b: [64(d), 16(b), 12]
            rows = {}
            for side, idxT in (("p", posT), ("n", negT)):
                rt = pp_pool.tile([DMAX, NB, LDW], i32, tag=f"rows_{side}")
                for b in range(NB):
                    nc.gpsimd.indirect_dma_start(
                        out=rt[:, b, :], out_offset=None, in_=locdeg_in[:],
                        in_offset=bass.IndirectOffsetOnAxis(ap=idxT[:, b:b + 1], axis=0))
                rows[side] = rt

            # ---- PE transposes to (b,t) layout ----
            loc_bt = {}
            for side in ("p", "n"):
                lf = pp_pool.tile([DMAX, P], f32, tag=f"locf_{side}")
                nc.vector.tensor_copy(
                    out=lf[:].rearrange("d (b t) -> d b t", b=NB),
                    in_=rows[side][:, :, 0:8])
                ps = ps_pool.tile([P, DMAX], f32, tag="ps_small")
                nc.tensor.transpose(out=ps[:], in_=lf[:], identity=ident64[:])
                li = pp_pool.tile([P, DMAX], i32, tag=f"loc_bt_{side}")
                nc.vector.tensor_copy(out=li[:], in_=ps[:])
                loc_bt[side] = li

            # deg2 columns stacked [64, 32] f32 -> transpose -> [32, 64]
            degs = pp_pool.tile([DMAX, 2 * NB], f32)
            nc.vector.tensor_copy(out=degs[:, 0:NB], in_=rows["p"][:, :, 8])
            nc.vector.tensor_copy(out=degs[:, NB:2 * NB], in_=rows["n"][:, :, 8])
            degsT_ps = ps_pool.tile([2 * NB, DMAX], f32, tag="ps_small")
            nc.tensor.transpose(out=degsT_ps[:], in_=degs[:], identity=ident64[:])
            degsT = pp_pool.tile([2 * NB, DMAX], f32)
            nc.vector.tensor_copy(out=degsT[:], in_=degsT_ps[:])

            # replicate b -> (b,t): deg2 side reps [128, 64]
            deg2_rep = {}
            for qi, side in ((0, "p"), (1, "n")):
                psd = ps_pool.tile([P, DMAX], f32, tag="ps_small")
                nc.tensor.matmul(out=psd[:], lhsT=trepq[:, qi * P:(qi + 1) * P],
                                 rhs=degsT[:], start=True, stop=True)
                deg2_rep[side] = psd

            # ---- mask, iv, U, W per side ----
            mask_f = pp_pool.tile([P, DMAX], f32)
            nc.vector.tensor_scalar(out=mask_f[:], in0=pos_bt[:], scalar1=0,
                                    scalar2=None, op0=A.is_ge)

            U, W = {}, {}
            for side in ("p", "n"):
                # X = sta ^ loc ; S = bitlen(X)
                X = pp_pool.tile([P, DMAX], i32, tag=f"X_{side}")
                nc.vector.tensor_tensor(out=X[:], in0=loc_bt[side][:],
                                        in1=sta_loc[:].to_broadcast([P, DMAX]),
                                        op=A.bitwise_xor)
                Xf = pp_pool.tile([P, DMAX], f32, tag=f"Xf_{side}")
                nc.scalar.copy(out=Xf[:], in_=X[:])
                e_t = pp_pool.tile([P, DMAX], i32, tag=f"e_{side}")
                nc.vector.tensor_scalar(out=e_t[:], in0=Xf[:].bitcast(i32),
                                        scalar1=23, scalar2=None,
                                        op0=A.logical_shift_right)
                srelu_scale = 1.0
                if fix8192:
                    # reference lut[8192] is 13 (f32 log2 artifact), not 14
                    e_t2 = pp_pool.tile([P, DMAX], i32, tag=f"e2_{side}")
                    nc.vector.scalar_tensor_tensor(
                        out=e_t2[:], in0=Xf[:], scalar=8192.0, in1=e_t[:],
                        op0=A.is_equal, op1=A.subtract)
                    e_t = e_t2
                    srelu_scale = -1.0
                S = pp_pool.tile([P, DMAX], f32, tag=f"S_{side}")
                nc.scalar.activation(out=S[:], in_=e_t[:], func=ACT.Relu,
                                     bias=-126.0, scale=srelu_scale)
                # sumS over t (within b), then replicate back
                s16 = ps_pool.tile([NB, DMAX], f32, tag="ps_small")
                nc.tensor.matmul(out=s16[:], lhsT=tsum[:], rhs=S[:],
                                 start=True, stop=True)
                s16s = pp_pool.tile([NB, DMAX], f32, tag=f"s16_{side}")
                nc.vector.tensor_copy(out=s16s[:], in_=s16[:])
                srep = ps_pool.tile([P, DMAX], f32, tag="ps_small")
                nc.tensor.matmul(out=srep[:], lhsT=trep[:], rhs=s16s[:],
                                 start=True, stop=True)
                A_t = pp_pool.tile([P, DMAX], f32, tag=f"A_{side}")
                nc.vector.tensor_tensor(out=A_t[:], in0=srep[:], in1=S[:],
                                        op=A.subtract)
                # iv = 1/ln((deg1+1)(deg2+1))
                lp = pp_pool.tile([P, DMAX], f32, tag=f"lp_{side}")
                nc.vector.tensor_scalar(out=lp[:], in0=deg2_rep[side][:],
                                        scalar1=1.0, scalar2=deg1p1[:, :1],
                                        op0=A.add, op1=A.mult)
                lga = pp_pool.tile([P, DMAX], f32, tag=f"lga_{side}")
                nc.scalar.activation(out=lga[:], in_=lp[:], func=ACT.Ln,
                                     bias=0.0, scale=1.0)
                iv = pp_pool.tile([P, DMAX], f32, tag=f"iv_{side}")
                nc.vector.reciprocal(out=iv[:], in_=lga[:])
                # U = mask*iv/8
                Ut = pp_pool.tile([P, DMAX], f32, tag=f"U_{side}")
                nc.vector.scalar_tensor_tensor(out=Ut[:], in0=mask_f[:],
                                               scalar=0.125, in1=iv[:],
                                               op0=A.mult, op1=A.mult)
                U[side] = Ut
                # W
                Wt = pp_pool.tile([P, DMAX], f32, tag=f"W_{side}")
                if side == "p":
                    t1 = pp_pool.tile([P, DMAX], f32, tag="w_t1_p")
                    nc.vector.scalar_tensor_tensor(out=t1[:], in0=A_t[:],
                                                   scalar=0.125, in1=mask_f[:],
                                                   op0=A.mult, op1=A.mult)
                    nc.vector.scalar_tensor_tensor(out=Wt[:], in0=t1[:],
                                                   scalar=EPS, in1=iv[:],
                                                   op0=A.add, op1=A.mult)
                else:
                    t1 = pp_pool.tile([P, DMAX], f32, tag="w_t1_n")
                    nc.vector.tensor_scalar(out=t1[:], in0=A_t[:], scalar1=0.125,
                                            scalar2=1000.0, op0=A.mult,
                                            op1=A.subtract)
                    t2 = pp_pool.tile([P, DMAX], f32, tag="w_t2_n")
                    nc.vector.tensor_tensor(out=t2[:], in0=t1[:], in1=mask_f[:],
                                            op=A.mult)
                    nc.vector.scalar_tensor_tensor(out=Wt[:], in0=t2[:],
                                                   scalar=1000.0 + EPS, in1=iv[:],
                                                   op0=A.add, op1=A.mult)
                W[side] = Wt

            # ---- candidate values cncv [128, 226] ----
            cncv = pp_pool.tile([P, NCAND], i32)
            nc.vector.tensor_tensor(out=cncv[:, 0:NCAND - 1], in0=rmask_bt[:],
                                    in1=flip_rep[:], op=A.bitwise_xor)
            nc.vector.memset(cncv[:, NCAND - 1:NCAND], 0)
            nc.vector.tensor_tensor(out=cncv[:], in0=cncv[:],
                                    in1=sta_loc[:].to_broadcast([P, NCAND]),
                                    op=A.bitwise_xor)
            cncf = pp_pool.tile([P, NCAND], f32)
            nc.vector.tensor_copy(out=cncf[:], in_=cncv[:])

            # ---- big loop ----
            posred = pp_pool.tile([P, NCAND], f32)
            negred = pp_pool.tile([P, NCAND], f32)
            red_parts = []
            c0 = 0
            for cc in CCHUNKS:
                cs = slice(c0, c0 + cc)
                cv_b = cncv[:, cs].unsqueeze(2).to_broadcast([P, cc, DMAX])
                for side, red in (("p", posred), ("n", negred)):
                    lb_b = loc_bt[side][:].unsqueeze(1).to_broadcast([P, cc, DMAX])
                    U_b = U[side][:].unsqueeze(1).to_broadcast([P, cc, DMAX])
                    W_b = W[side][:].unsqueeze(1).to_broadcast([P, cc, DMAX])

                    y = big_pool.tile([P, cc, DMAX], i32, tag="tA")
                    nc.vector.tensor_tensor(out=y[:], in0=cv_b, in1=lb_b,
                                            op=A.bitwise_xor)
                    yf = big_pool.tile([P, cc, DMAX], f32, tag="tB")
                    nc.scalar.copy(out=yf[:], in_=y[:])
                    e_b = big_pool.tile([P, cc, DMAX], i32, tag="tC")
                    nc.vector.tensor_scalar(out=e_b[:], in0=yf[:].bitcast(i32),
                                            scalar1=23, scalar2=None,
                                            op0=A.logical_shift_right)
                    relu_scale = 1.0
                    if fix8192:
                        e_b2 = big_pool.tile([P, cc, DMAX], i32, tag="tD")
                        nc.vector.scalar_tensor_tensor(
                            out=e_b2[:], in0=yf[:], scalar=8192.0, in1=e_b[:],
                            op0=A.is_equal, op1=A.subtract)
                        e_b = e_b2
                        relu_scale = -1.0
                    Lf = big_pool.tile([P, cc, DMAX], f32, tag="tE")
                    nc.scalar.activation(out=Lf[:], in_=e_b[:], func=ACT.Relu,
                                         bias=-126.0, scale=relu_scale)
                    t1 = big_pool.tile([P, cc, DMAX], f32, tag="tF")
                    nc.vector.tensor_tensor(out=t1[:], in0=Lf[:], in1=U_b,
                                            op=A.mult)
                    aa = big_pool.tile([P, cc, DMAX], f32, tag="tG")
                    nc.vector.tensor_tensor(out=aa[:], in0=t1[:], in1=W_b,
                                            op=A.add)
                    q = big_pool.tile([P, cc, DMAX], f32, tag="tA")
                    nc.scalar.activation(out=q[:], in_=aa[:], func=ACT.Square)
                    lnp = big_pool.tile([P, cc, DMAX], f32, tag="tB")
                    nc.scalar.activation(out=lnp[:], in_=q[:], func=ACT.Ln,
                                         bias=1.0, scale=1.0)
                    if side == "p":
                        r = lnp
                    else:
                        ln2 = big_pool.tile([P, cc, DMAX], f32, tag="tC")
                        nc.scalar.activation(out=ln2[:], in_=q[:], func=ACT.Ln,
                                             bias=0.1, scale=1.0 + EPS)
                        r = big_pool.tile([P, cc, DMAX], f32, tag="tD")
                        nc.vector.tensor_tensor(out=r[:], in0=lnp[:], in1=ln2[:],
                                                op=A.subtract)
                    rc = red_pool.tile([P, cc], f32, tag="tred")
                    nc.vector.tensor_reduce(out=rc[:], in_=r[:],
                                            axis=mybir.AxisListType.X, op=A.add)
                    red_parts.append((red, cs, rc))
                c0 += cc
            for red, cs, rc in red_parts:
                nc.vector.tensor_copy(out=red[:, cs], in_=rc[:])

            # ---- losses, tie-aware argmin, selection ----
            tsum2 = pp_pool.tile([P, NCAND], f32)
            nc.vector.tensor_tensor(out=tsum2[:], in0=posred[:], in1=negred[:],
                                    op=A.add)
            total = pp_pool.tile([P, NCAND], f32)
            nc.vector.tensor_scalar(out=total[:], in0=tsum2[:],
                                    scalar1=invlg[:, :1], scalar2=None, op0=A.mult)
            minv = pp_pool.tile([P, 1], f32)
            nc.vector.tensor_reduce(out=minv[:], in_=total[:],
                                    axis=mybir.AxisListType.X, op=A.min)
            eqm = pp_pool.tile([P, NCAND], i32)
            nc.vector.tensor_scalar(out=eqm[:], in0=total[:], scalar1=minv[:, :1],
                                    scalar2=None, op0=A.is_equal)
            bigc = pp_pool.tile([P, 1], f32)
            nc.vector.memset(bigc[:], BIGF)
            score = pp_pool.tile([P, NCAND], f32)
            nc.vector.select(out=score[:], mask=eqm[:], on_true=pp_rep[:],
                             on_false=bigc[:].to_broadcast([P, NCAND]))
            pstar = pp_pool.tile([P, 1], f32)
            nc.vector.tensor_reduce(out=pstar[:], in_=score[:],
                                    axis=mybir.AxisListType.X, op=A.min)
            mask2 = pp_pool.tile([P, NCAND], f32)
            nc.vector.tensor_scalar(out=mask2[:], in0=score[:], scalar1=pstar[:, :1],
                                    scalar2=None, op0=A.is_equal)

            sel = pp_pool.tile([P, 8], f32)
            nc.vector.memset(sel[:], 0.0)
            scrap = pp_pool.tile([P, NCAND], f32)
            for col, val in ((0, posred), (1, negred), (3, cncf)):
                nc.vector.scalar_tensor_tensor(
                    out=scrap[:], in0=mask2[:], scalar=1.0, in1=val[:],
                    op0=A.mult, op1=A.mult, accum_out=sel[:, col:col + 1])
            nc.sync.dma_start(out=out_sel[:], in_=sel[:])

    _legalize_waits(nc)
    return nc


def _build_launch_B():
    import concourse.bass as bass
    import concourse.mybir as mybir
    from concourse.tile import TileContext

    i32 = mybir.dt.int32
    nc = bass.Bass()
    dp = nc.declare_dram_parameter
    shard_in = dp("shard_in", [SHARD, TP], i32, isOutput=False)
    scat_idx_in = dp("scat_idx", [P, 1], i32, isOutput=False)
    scat_val_in = dp("scat_val", [P, TP], i32, isOutput=False)
    shard_out = dp("shard_out", [SHARD, TP], i32, isOutput=True)

    with TileContext(nc) as tc:
        with tc.tile_pool(name="sbuf", bufs=1) as pool:
            nc.sync.dma_start(out=shard_out[:], in_=shard_in[:])
            sidx = pool.tile([P, 1], i32)
            nc.sync.dma_start(out=sidx[:], in_=scat_idx_in[:])
            sval = pool.tile([P, TP], i32)
            nc.sync.dma_start(out=sval[:], in_=scat_val_in[:])
            nc.gpsimd.indirect_dma_start(
                out=shard_out[:],
                out_offset=bass.IndirectOffsetOnAxis(ap=sidx[:, :1], axis=0),
                in_=sval[:], in_offset=None,
                bounds_check=SHARD - 1, oob_is_err=False)
    _legalize_waits(nc)
    return nc


# --------------------------------------------------------------------------
# host-side driver
# --------------------------------------------------------------------------

def _consts():
    flip_jk = np.repeat((1 << np.arange(H)).astype(np.int32), K)       # [225]
    tsum = np.zeros((P, NB), np.float32)
    tsum[np.arange(P), np.arange(P) // TP] = 1.0
    trep = np.zeros((NB, P), np.float32)
    trep[np.arange(P) // TP, np.arange(P)] = 1.0
    trepq = np.zeros((2 * NB, 2 * P), np.float32)
    trepq[np.arange(P) // TP, np.arange(P)] = 1.0                 # q0 block
    trepq[NB + np.arange(P) // TP, P + np.arange(P)] = 1.0        # q1 block
    tsel = np.zeros((P, LDW), np.int32)
    tsel[np.arange(P), np.arange(P) % TP] = 1
    ident64 = np.eye(DMAX, dtype=np.float32)
    return flip_jk, tsum, trep, trepq, tsel, ident64


def _wrap(idx):
    return np.where(idx < 0, idx + NNODES, idx).astype(np.int32)


def _numpy_ref(lut, sta_ind, locations, degree, pos_ind, neg_ind, random_masks, perm):
    """Exact numpy fallback mirroring reference.py (used only if lut is not
    the bit-length table)."""
    mask = pos_ind != -1
    lg = mask.sum(axis=1).astype(np.float32)
    sta_loc = locations[sta_ind]
    pos_loc = np.where(mask[:, :, None], locations[np.where(mask, pos_ind, 0)], -1)
    neg_loc = np.where(mask[:, :, None], locations[np.where(mask, neg_ind, 0)], -1)
    flip = (1 << np.arange(H, dtype=np.int32))[None, :, None]
    flipped = sta_loc[:, None, :] ^ flip
    cand = (flipped[:, :, None, :] ^ random_masks).reshape(BATCH, H * K, TP)
    cnc_loc = np.concatenate([cand, sta_loc[:, None, :]], axis=1)[:, perm, :]
    m1 = mask[:, :, None]
    m2 = mask[:, :, None, None]

    def dist(xor, m):
        return np.where(m, lut[np.where(m, xor, 0)], -1.0).astype(np.float32)

    dsp = dist(sta_loc[:, None, :] ^ pos_loc, m1)
    dsps = dsp.sum(axis=-1)
    dsn = dist(sta_loc[:, None, :] ^ neg_loc, m1)
    dsns = dsn.sum(axis=-1)
    dpc = dist(cnc_loc[:, None, :, :] ^ pos_loc[:, :, None, :], m2)
    dnc = dist(cnc_loc[:, None, :, :] ^ neg_loc[:, :, None, :], m2)
    dnp_ = np.where(m2, (dpc - dsp[:, :, None, :] + dsps[:, :, None, None]) / TP, 0.0)
    dnn = np.where(m2, (dnc - dsn[:, :, None, :] + dsns[:, :, None, None]) / TP, 1000.0)

    def p_(dis, ig2):
        deg1 = degree[sta_ind][:, None]
        ig2w = np.where(ig2 < 0, ig2 + NNODES, ig2)
        deg2 = degree[ig2w]
        log_ap = np.log(((deg1 + 1) * (deg2 + 1)).astype(np.float32))
        aaq = (dis + EPS) / log_ap[:, :, None, None]
        return 1.0 / (1.0 + aaq ** GAMMA / ALPHA)

    pos_loss = -np.sum(np.log(p_(dnp_, pos_ind)), axis=1) / lg[:, None, None]
    neg_loss = -np.sum(np.log(1.0 + EPS - p_(dnn, neg_ind)), axis=1) / lg[:, None, None]
    total = POS_RATIO * pos_loss + neg_loss
    index = np.argmin(total, axis=1)
    sel_loc = np.take_along_axis(cnc_loc, index[:, None, :], axis=1)[:, 0, :]
    new_loc = locations.copy()
    new_loc[sta_ind] = sel_loc

    def gsel(x):
        return np.take_along_axis(x, index[:, None, :], axis=1)[:, 0, :]

    return (np.stack([gsel(total).mean(), gsel(pos_loss).mean(),
                      gsel(neg_loss).mean()]).astype(np.float32),
            new_loc.astype(np.int32))


def _ensure_ntff_hook():
    """Register the NTFF profiling hook this image's antenv lacks, and stub
    out the artifact upload (no bucket access here)."""
    import sys, types
    if "antenv.axon_hooks" not in sys.modules:
        mod = types.ModuleType("antenv.axon_hooks")
        holder = {}
        mod.set_axon_ntff_profile_hook = lambda h: holder.__setitem__("h", h)
        mod.get_axon_ntff_profile_hook = lambda: holder.get("h")
        try:
            from trn_agent_boot.trn_boot import _ntff_profile_via_ctypes
            hook = _ntff_profile_via_ctypes("/opt/axon/libaxon_pjrt.so")
            if hook is not None:
                holder["h"] = hook
        except Exception:
            pass
        sys.modules["antenv.axon_hooks"] = mod
        import antenv
        antenv.axon_hooks = mod
    import concourse.bass_utils as bu
    bu.upload_artifacts = lambda tmpdir: f"local:{tmpdir}"


def kernel(lut, sta_ind, locations, degree, pos_ind, neg_ind, random_masks, perm):
    lut = np.asarray(lut, np.float32)
    sta_ind = np.asarray(sta_ind, np.int32)
    locations = np.asarray(locations, np.int32)
    degree = np.asarray(degree, np.int32)
    pos_ind = np.asarray(pos_ind, np.int32)
    neg_ind = np.asarray(neg_ind, np.int32)
    random_masks = np.asarray(random_masks, np.int32)
    perm = np.asarray(perm, np.int32)

    tab = _bitlen_table()
    if np.array_equal(lut, tab):
        fix8192 = False
    else:
        d = tab - lut
        only8192 = (d[NLOC // 4] == 1.0) and (np.count_nonzero(d) == 1)
        if only8192:
            fix8192 = True
        else:
            return _numpy_ref(lut, sta_ind, locations, degree, pos_ind, neg_ind,
                              random_masks, perm)

    from concourse.bass_utils import run_bass_kernel_spmd

    key = ("A", fix8192)
    if key not in _CACHE:
        _CACHE[key] = _build_launch_A(fix8192)
        _CACHE["B"] = _build_launch_B()
    _CACHE["A"] = _CACHE[key]

    flip_jk, tsum, trep, trepq, tsel, ident64 = _consts()
    pp = np.empty(NCAND, np.float32)
    pp[perm] = np.arange(NCAND, dtype=np.float32)
    pp_rep = np.tile(pp[None, :], (P, 1))
    flip_rep = np.tile(flip_jk[None, :], (P, 1)).astype(np.int32)

    locdeg = np.zeros((NNODES, LDW), np.int32)
    locdeg[:, 0:TP] = locations
    locdeg[:, 8] = degree

    in_maps_A = []
    for m in range(NCORES):
        bs = slice(m * NB, (m + 1) * NB)
        pos_l, neg_l = pos_ind[bs], neg_ind[bs]
        in_maps_A.append(dict(
            locdeg=locdeg,
            sta_bt=np.repeat(sta_ind[bs], TP).reshape(P, 1).astype(np.int32),
            posT=np.ascontiguousarray(_wrap(pos_l).T),
            negT=np.ascontiguousarray(_wrap(neg_l).T),
            pos_bt=np.repeat(pos_l, TP, axis=0).astype(np.int32),
            rmask_bt=np.ascontiguousarray(
                random_masks[bs].transpose(0, 3, 1, 2).reshape(P, H * K)).astype(np.int32),
            flip_rep=flip_rep, pp_rep=pp_rep, ident64=ident64,
            tsum=tsum, trep=trep, trepq=trepq, tsel=tsel,
        ))
    import os, tempfile
    trace = bool(os.environ.get("KERNEL_TRACE"))
    tkw = {}
    global LAST_TRACE_DIR_A, LAST_TRACE_DIR_B
    if trace:
        _ensure_ntff_hook()
        LAST_TRACE_DIR_A = tempfile.mkdtemp(prefix="ktrA_")
        tkw = dict(trace=True, tmpdir=LAST_TRACE_DIR_A)
    resA = run_bass_kernel_spmd(_CACHE["A"], in_maps_A, list(range(NCORES)), **tkw)
    global LAST_EXEC_NS_A
    LAST_EXEC_NS_A = resA.exec_time_ns
    sel = np.stack([resA.results[m]["out_sel"] for m in range(NCORES)])  # [8,128,8]

    sel_bt = sel.reshape(NCORES, NB, TP, 8).reshape(BATCH, TP, 8)
    lg = (pos_ind != -1).sum(axis=1).astype(np.float32)                  # [128]
    pos_sel = sel_bt[:, :, 0] / lg[:, None]
    neg_sel = sel_bt[:, :, 1] / lg[:, None]
    tl = float((pos_sel + neg_sel).mean())
    pl = float(pos_sel.mean())
    nl = float(neg_sel.mean())
    sel_loc = np.rint(sel_bt[:, :, 3]).astype(np.int32)                  # [128, 8]

    # launch B: shard copy + disjoint scatter
    in_maps_B = []
    shard_of = sta_ind // SHARD
    for m in range(NCORES):
        sidx = np.full((P, 1), 1 << 20, np.int32)
        sval = np.zeros((P, TP), np.int32)
        rows = np.where(shard_of == m)[0]
        sidx[:len(rows), 0] = sta_ind[rows] - m * SHARD
        sval[:len(rows)] = sel_loc[rows]
        in_maps_B.append(dict(
            shard_in=np.ascontiguousarray(locations[m * SHARD:(m + 1) * SHARD]),
            scat_idx=sidx, scat_val=sval,
        ))
    tkwB = {}
    if trace:
        LAST_TRACE_DIR_B = tempfile.mkdtemp(prefix="ktrB_")
        tkwB = dict(trace=True, tmpdir=LAST_TRACE_DIR_B)
    resB = run_bass_kernel_spmd(_CACHE["B"], in_maps_B, list(range(NCORES)), **tkwB)
    global LAST_EXEC_NS_B, LAST_EXEC_NS
    LAST_EXEC_NS_B = resB.exec_time_ns
    if LAST_EXEC_NS_A is not None and LAST_EXEC_NS_B is not None:
        LAST_EXEC_NS = LAST_EXEC_NS_A + LAST_EXEC_NS_B
    new_loc = np.concatenate(
        [resB.results[m]["shard_out"] for m in range(NCORES)], axis=0)

    return (np.stack([tl, pl, nl]).astype(np.float32), new_loc.astype(np.int32))


# revision 29
# speedup vs baseline: 1.2777x; 1.2524x over previous
"""CritiGraph update-step kernel for 8 Trainium2 NeuronCores (Bass/Tile).

Contract: kernel(**inputs) takes the FULL (unsharded) inputs of
reference.setup_inputs() and returns the FULL output
(np.stack([tl, pl, nl]), new_locations).

Strategy:
 - Data-parallel over batch: core m computes batch rows [16m, 16m+16).
 - Device layout: SBUF partitions = (b_local, t) = 16*8 = 128; free dims
   = (candidate, neighbor d). Both xor operands of the hot loop are
   rank-broadcast (stride-0) APs, so nothing big is ever materialized
   twice.
 - lut is the bit-length table (floor(log2(x))+1); computed on device
   arithmetically: int->f32 convert, exponent extract, Relu(e-126).
   Verified on host; falls back to a numpy reference if it mismatches.
 - Gathers of locations/degree rows are indirect DMAs from a host-packed
   locdeg [N, 12] array (cols 0-7 locations, col 8 degree) using wrapped
   indices; gathered in [64(d), (b,t)] layout, then one PE transpose per
   side -> [(b,t), d].
 - Launch B: each core copies its 12500-row locations shard DRAM->DRAM
   and indirect-scatters its assigned updated rows (disjoint per shard).
"""

import numpy as np

# ---- problem constants (hardcoded; kernel.py must be self-contained) ----
H, K, TP = 15, 15, 8
NLOC = 2 ** H                    # 32768
NNODES, BATCH, DMAX = 100000, 128, 64
NCAND = H * K + 1                # 226
EPS, GAMMA, ALPHA, POS_RATIO = 0.1, 2.0, 1.0, 1.0
NCORES = 8
NB = BATCH // NCORES             # 16 batch rows per core
SHARD = NNODES // NCORES         # 12500 location rows per core
P = 128
LDW = 12                         # locdeg row width (8 loc + 1 deg + 3 pad)
CCHUNKS = [29, 29, 29, 29, 29, 29, 29, 23]  # candidate chunks (sum = 226)
BIGF = 65536.0                   # tie-break sentinel (exact in f32)

_CACHE = {}
LAST_TRACE_DIR_A = None
LAST_TRACE_DIR_B = None
LAST_EXEC_NS = None
LAST_EXEC_NS_A = None
LAST_EXEC_NS_B = None


def _bitlen_table():
    xs = np.arange(NLOC)
    return np.where(xs == 0, 0.0,
                    np.floor(np.log2(np.maximum(xs, 1))) + 1.0).astype(np.float32)


# --------------------------------------------------------------------------
# device kernels
# --------------------------------------------------------------------------

def _reg_consts(nc, vals):
    import concourse.mybir as mybir
    for v in vals:
        t = nc.alloc_sbuf_tensor(f"const-float32-{v}", [128, 1], mybir.dt.float32)
        nc.gpsimd.memset(t.ap(), v)
        nc.const_aps.aps[(mybir.dt.float32, v)] = t.ap()


def _legalize_waits(nc, max_waits=1):
    """walrus CoreV3 codegen accepts only one sync-wait command per
    instruction; hoist extras onto preceding NoOps on the same engine."""
    import concourse.mybir as mybir
    n = 0
    for func in nc.m.functions:
        for bb in func.blocks:
            out = []
            for ins in bb.instructions:
                si = getattr(ins, "sync_info", None)
                waits = list(si.on_wait) if si is not None and si.on_wait else []
                if len(waits) > max_waits:
                    for w in waits[:-max_waits]:
                        out.append(mybir.InstNoOp(
                            name=f"{ins.name}-w{n}", engine=ins.engine,
                            ins=[], outs=[],
                            sync_info=mybir.SyncInfo(on_wait=[w], on_update=[])))
                        n += 1
                    si.on_wait = waits[-max_waits:]
                out.append(ins)
            bb.instructions = out
    return n


def _build_launch_A(fix8192):
    import concourse.bass as bass
    import concourse.mybir as mybir
    from concourse.tile import TileContext

    i32, f32 = mybir.dt.int32, mybir.dt.float32
    i16 = mybir.dt.int16
    A = mybir.AluOpType
    ACT = mybir.ActivationFunctionType

    nc = bass.Bass()
    _reg_consts(nc, [-126.0, 0.1, 1.0 + EPS])

    dp = nc.declare_dram_parameter
    locdeg_in = dp("locdeg", [NNODES, LDW], i32, isOutput=False)
    sta_bt_in = dp("sta_bt", [P, 1], i32, isOutput=False)       # repeat(sta,8)
    posT_in = dp("posT", [DMAX, NB], i32, isOutput=False)      # wrapped
    negT_in = dp("negT", [DMAX, NB], i32, isOutput=False)
    pos_bt_in = dp("pos_bt", [P, DMAX], i32, isOutput=False)    # raw (-1s)
    rmask_bt_in = dp("rmask_bt", [P, NCAND - 1], i32, isOutput=False)
    flip_in = dp("flip_rep", [P, NCAND - 1], i32, isOutput=False)
    pp_in = dp("pp_rep", [P, NCAND], f32, isOutput=False)
    ident_in = dp("ident64", [DMAX, DMAX], f32, isOutput=False)
    tsum_in = dp("tsum", [P, NB], f32, isOutput=False)
    trep_in = dp("trep", [NB, P], f32, isOutput=False)
    trepq_in = dp("trepq", [2 * NB, 2 * P], f32, isOutput=False)  # [q0|q1] stacked
    tsel_in = dp("tsel", [P, LDW], i32, isOutput=False)

    out_sel = dp("out_sel", [P, 8], f32, isOutput=True)

    with TileContext(nc) as tc:
        with (
            tc.tile_pool(name="persist", bufs=1) as pp_pool,
            tc.tile_pool(name="big", bufs=14) as big_pool,
            tc.tile_pool(name="psum", bufs=4, space="PSUM") as ps_pool,
        ):
            # ---- load small inputs ----
            def load(name, src, shape, dtype):
                t = pp_pool.tile(shape, dtype, tag=name)
                nc.sync.dma_start(out=t[:], in_=src[:])
                return t

            sta_bt = load("sta_bt", sta_bt_in, [P, 1], i32)
            posT = load("posT", posT_in, [DMAX, NB], i32)
            negT = load("negT", negT_in, [DMAX, NB], i32)
            pos_bt = load("pos_bt", pos_bt_in, [P, DMAX], i32)
            rmask_bt = load("rmask_bt", rmask_bt_in, [P, NCAND - 1], i32)
            flip_rep = load("flip_rep", flip_in, [P, NCAND - 1], i32)
            pp_rep = load("pp_rep", pp_in, [P, NCAND], f32)
            ident64 = load("ident64", ident_in, [DMAX, DMAX], f32)
            tsum = load("tsum", tsum_in, [P, NB], f32)
            trep = load("trep", trep_in, [NB, P], f32)
            trepq = load("trepq", trepq_in, [2 * NB, 2 * P], f32)
            tsel = load("tsel", tsel_in, [P, LDW], i32)

            # ---- gathers ----
            # sta row: locdeg[sta] -> [128, 12]
            sta_row = pp_pool.tile([P, LDW], i32)
            nc.gpsimd.indirect_dma_start(
                out=sta_row[:], out_offset=None, in_=locdeg_in[:],
                in_offset=bass.IndirectOffsetOnAxis(ap=sta_bt[:, :1], axis=0))
            # sta_loc[p] = sta_row[p, p%8] via one-hot tsel
            stp = pp_pool.tile([P, LDW], i32)
            nc.vector.tensor_tensor(out=stp[:], in0=sta_row[:], in1=tsel[:], op=A.mult)
            sta_loc = pp_pool.tile([P, 1], i32)
            with nc.allow_low_precision(reason="int32 one-hot reduce is exact"):
                nc.vector.tensor_reduce(out=sta_loc[:], in_=stp[:],
                                        axis=mybir.AxisListType.X, op=A.add)
            deg1f = pp_pool.tile([P, 1], f32)
            nc.vector.tensor_copy(out=deg1f[:], in_=sta_row[:, 8:9])
            deg1p1 = pp_pool.tile([P, 1], f32)
            nc.vector.tensor_scalar(out=deg1p1[:], in0=deg1f[:], scalar1=1.0,
                                    scalar2=None, op0=A.add)
            invlg = pp_pool.tile([P, 1], f32)
            nc.vector.reciprocal(out=invlg[:], in_=deg1f[:])

            # neighbor rows, transposed-gather per b: [64(d), 16(b), 12]
            rows = {}
            for side, idxT in (("p", posT), ("n", negT)):
                rt = pp_pool.tile([DMAX, NB, LDW], i32, tag=f"rows_{side}")
                for b in range(NB):
                    nc.gpsimd.indirect_dma_start(
                        out=rt[:, b, :], out_offset=None, in_=locdeg_in[:],
                        in_offset=bass.IndirectOffsetOnAxis(ap=idxT[:, b:b + 1], axis=0))
                rows[side] = rt

            # ---- PE transposes to (b,t) layout ----
            loc_bt = {}
            for side in ("p", "n"):
                lf = pp_pool.tile([DMAX, P], f32, tag=f"locf_{side}")
                nc.vector.tensor_copy(
                    out=lf[:].rearrange("d (b t) -> d b t", b=NB),
                    in_=rows[side][:, :, 0:8])
                ps = ps_pool.tile([P, DMAX], f32, tag="ps_small")
                nc.tensor.transpose(out=ps[:], in_=lf[:], identity=ident64[:])
                li = pp_pool.tile([P, DMAX], i32, tag=f"loc_bt_{side}")
                nc.vector.tensor_copy(out=li[:], in_=ps[:])
                loc_bt[side] = li

            # deg2 columns stacked [64, 32] f32 -> transpose -> [32, 64]
            degs = pp_pool.tile([DMAX, 2 * NB], f32)
            nc.vector.tensor_copy(out=degs[:, 0:NB], in_=rows["p"][:, :, 8])
            nc.vector.tensor_copy(out=degs[:, NB:2 * NB], in_=rows["n"][:, :, 8])
            degsT_ps = ps_pool.tile([2 * NB, DMAX], f32, tag="ps_small")
            nc.tensor.transpose(out=degsT_ps[:], in_=degs[:], identity=ident64[:])
            degsT = pp_pool.tile([2 * NB, DMAX], f32)
            nc.vector.tensor_copy(out=degsT[:], in_=degsT_ps[:])

            # replicate b -> (b,t): deg2 side reps [128, 64]
            deg2_rep = {}
            for qi, side in ((0, "p"), (1, "n")):
                psd = ps_pool.tile([P, DMAX], f32, tag="ps_small")
                nc.tensor.matmul(out=psd[:], lhsT=trepq[:, qi * P:(qi + 1) * P],
                                 rhs=degsT[:], start=True, stop=True)
                deg2_rep[side] = psd

            # ---- mask, iv, U, W per side ----
            mask_f = pp_pool.tile([P, DMAX], f32)
            nc.vector.tensor_scalar(out=mask_f[:], in0=pos_bt[:], scalar1=0,
                                    scalar2=None, op0=A.is_ge)

            U, W = {}, {}
            for side in ("p", "n"):
                # X = sta ^ loc ; S = bitlen(X)
                X = pp_pool.tile([P, DMAX], i32, tag=f"X_{side}")
                nc.vector.tensor_tensor(out=X[:], in0=loc_bt[side][:],
                                        in1=sta_loc[:].to_broadcast([P, DMAX]),
                                        op=A.bitwise_xor)
                Xf = pp_pool.tile([P, DMAX], f32, tag=f"Xf_{side}")
                nc.scalar.copy(out=Xf[:], in_=X[:])
                e_t = pp_pool.tile([P, DMAX], i32, tag=f"e_{side}")
                nc.vector.tensor_scalar(out=e_t[:], in0=Xf[:].bitcast(i32),
                                        scalar1=23, scalar2=None,
                                        op0=A.logical_shift_right)
                srelu_scale = 1.0
                if fix8192:
                    # reference lut[8192] is 13 (f32 log2 artifact), not 14
                    e_t2 = pp_pool.tile([P, DMAX], i32, tag=f"e2_{side}")
                    nc.vector.scalar_tensor_tensor(
                        out=e_t2[:], in0=Xf[:], scalar=8192.0, in1=e_t[:],
                        op0=A.is_equal, op1=A.subtract)
                    e_t = e_t2
                    srelu_scale = -1.0
                S = pp_pool.tile([P, DMAX], f32, tag=f"S_{side}")
                nc.scalar.activation(out=S[:], in_=e_t[:], func=ACT.Relu,
                                     bias=-126.0, scale=srelu_scale)
                # sumS over t (within b), then replicate back
                s16 = ps_pool.tile([NB, DMAX], f32, tag="ps_small")
                nc.tensor.matmul(out=s16[:], lhsT=tsum[:], rhs=S[:],
                                 start=True, stop=True)
                s16s = pp_pool.tile([NB, DMAX], f32, tag=f"s16_{side}")
                nc.vector.tensor_copy(out=s16s[:], in_=s16[:])
                srep = ps_pool.tile([P, DMAX], f32, tag="ps_small")
                nc.tensor.matmul(out=srep[:], lhsT=trep[:], rhs=s16s[:],
                                 start=True, stop=True)
                A_t = pp_pool.tile([P, DMAX], f32, tag=f"A_{side}")
                nc.vector.tensor_tensor(out=A_t[:], in0=srep[:], in1=S[:],
                                        op=A.subtract)
                # iv = 1/ln((deg1+1)(deg2+1))
                lp = pp_pool.tile([P, DMAX], f32, tag=f"lp_{side}")
                nc.vector.tensor_scalar(out=lp[:], in0=deg2_rep[side][:],
                                        scalar1=1.0, scalar2=deg1p1[:, :1],
                                        op0=A.add, op1=A.mult)
                lga = pp_pool.tile([P, DMAX], f32, tag=f"lga_{side}")
                nc.scalar.activation(out=lga[:], in_=lp[:], func=ACT.Ln,
                                     bias=0.0, scale=1.0)
                iv = pp_pool.tile([P, DMAX], f32, tag=f"iv_{side}")
                nc.vector.reciprocal(out=iv[:], in_=lga[:])
                # U = mask*iv/8
                Ut = pp_pool.tile([P, DMAX], f32, tag=f"U_{side}")
                nc.vector.scalar_tensor_tensor(out=Ut[:], in0=mask_f[:],
                                               scalar=0.125, in1=iv[:],
                                               op0=A.mult, op1=A.mult)
                U[side] = Ut
                # W
                Wt = pp_pool.tile([P, DMAX], f32, tag=f"W_{side}")
                if side == "p":
                    t1 = pp_pool.tile([P, DMAX], f32, tag="w_t1_p")
                    nc.vector.scalar_tensor_tensor(out=t1[:], in0=A_t[:],
                                                   scalar=0.125, in1=mask_f[:],
                                                   op0=A.mult, op1=A.mult)
                    nc.vector.scalar_tensor_tensor(out=Wt[:], in0=t1[:],
                                                   scalar=EPS, in1=iv[:],
                                                   op0=A.add, op1=A.mult)
                else:
                    t1 = pp_pool.tile([P, DMAX], f32, tag="w_t1_n")
                    nc.vector.tensor_scalar(out=t1[:], in0=A_t[:], scalar1=0.125,
                                            scalar2=1000.0, op0=A.mult,
                                            op1=A.subtract)
                    t2 = pp_pool.tile([P, DMAX], f32, tag="w_t2_n")
                    nc.vector.tensor_tensor(out=t2[:], in0=t1[:], in1=mask_f[:],
                                            op=A.mult)
                    nc.vector.scalar_tensor_tensor(out=Wt[:], in0=t2[:],
                                                   scalar=1000.0 + EPS, in1=iv[:],
                                                   op0=A.add, op1=A.mult)
                W[side] = Wt

            # ---- candidate values cncv [128, 226] ----
            cncv = pp_pool.tile([P, NCAND], i32)
            nc.vector.tensor_tensor(out=cncv[:, 0:NCAND - 1], in0=rmask_bt[:],
                                    in1=flip_rep[:], op=A.bitwise_xor)
            nc.vector.memset(cncv[:, NCAND - 1:NCAND], 0)
            nc.vector.tensor_tensor(out=cncv[:], in0=cncv[:],
                                    in1=sta_loc[:].to_broadcast([P, NCAND]),
                                    op=A.bitwise_xor)
            cncf = pp_pool.tile([P, NCAND], f32)
            nc.vector.tensor_copy(out=cncf[:], in_=cncv[:])

            # ---- big loop: software-pipelined wavefront over chunk-sides ----
            posred = pp_pool.tile([P, NCAND], f32)
            negred = pp_pool.tile([P, NCAND], f32)

            sides = []
            c0 = 0
            for cc in CCHUNKS:
                for side in ("p", "n"):
                    sides.append((c0, cc, side))
                c0 += cc

            def make_stages(c0, cc, side):
                cs = slice(c0, c0 + cc)
                cv_b = cncv[:, cs].unsqueeze(2).to_broadcast([P, cc, DMAX])
                lb_b = loc_bt[side][:].unsqueeze(1).to_broadcast([P, cc, DMAX])
                U_b = U[side][:].unsqueeze(1).to_broadcast([P, cc, DMAX])
                W_b = W[side][:].unsqueeze(1).to_broadcast([P, cc, DMAX])
                red = posred if side == "p" else negred
                st = {}

                def s0():
                    st["y"] = big_pool.tile([P, cc, DMAX], i32, tag="ring", name=f"ring_y_{c0}_{side}")
                    nc.vector.tensor_tensor(out=st["y"][:], in0=cv_b, in1=lb_b,
                                            op=A.bitwise_xor)

                def s1():
                    st["yf"] = big_pool.tile([P, cc, DMAX], f32, tag="ring", name=f"ring_yf_{c0}_{side}")
                    nc.scalar.copy(out=st["yf"][:], in_=st["y"][:])

                def s2():
                    st["e"] = big_pool.tile([P, cc, DMAX], i32, tag="ring", name=f"ring_e_{c0}_{side}")
                    nc.vector.tensor_scalar(out=st["e"][:],
                                            in0=st["yf"][:].bitcast(i32),
                                            scalar1=23, scalar2=None,
                                            op0=A.logical_shift_right)

                def s3():
                    if fix8192:
                        st["e2"] = big_pool.tile([P, cc, DMAX], i32, tag="ring", name=f"ring_e2_{c0}_{side}")
                        nc.vector.scalar_tensor_tensor(
                            out=st["e2"][:], in0=st["yf"][:], scalar=8192.0,
                            in1=st["e"][:], op0=A.is_equal, op1=A.subtract)
                    else:
                        st["e2"] = st["e"]

                def s4():
                    st["Lf"] = big_pool.tile([P, cc, DMAX], f32, tag="ring", name=f"ring_Lf_{c0}_{side}")
                    nc.scalar.activation(out=st["Lf"][:], in_=st["e2"][:],
                                         func=ACT.Relu, bias=-126.0,
                                         scale=-1.0 if fix8192 else 1.0)

                def s5():
                    st["t1"] = big_pool.tile([P, cc, DMAX], f32, tag="ring", name=f"ring_t1_{c0}_{side}")
                    nc.vector.tensor_tensor(out=st["t1"][:], in0=st["Lf"][:],
                                            in1=U_b, op=A.mult)

                def s6():
                    st["aa"] = big_pool.tile([P, cc, DMAX], f32, tag="ring", name=f"ring_aa_{c0}_{side}")
                    nc.vector.tensor_tensor(out=st["aa"][:], in0=st["t1"][:],
                                            in1=W_b, op=A.add)

                def s7():
                    st["q"] = big_pool.tile([P, cc, DMAX], f32, tag="ring", name=f"ring_q_{c0}_{side}")
                    nc.scalar.activation(out=st["q"][:], in_=st["aa"][:],
                                         func=ACT.Square)

                def s8():
                    st["lnp"] = big_pool.tile([P, cc, DMAX], f32, tag="ring", name=f"ring_lnp_{c0}_{side}")
                    nc.scalar.activation(out=st["lnp"][:], in_=st["q"][:],
                                         func=ACT.Ln, bias=1.0, scale=1.0)

                def s9():
                    if side == "n":
                        st["ln2"] = big_pool.tile([P, cc, DMAX], f32, tag="ring", name=f"ring_ln2_{c0}_{side}")
                        nc.scalar.activation(out=st["ln2"][:], in_=st["q"][:],
                                             func=ACT.Ln, bias=0.1,
                                             scale=1.0 + EPS)

                def s10():
                    if side == "n":
                        st["r"] = big_pool.tile([P, cc, DMAX], f32, tag="ring", name=f"ring_r_{c0}_{side}")
                        nc.vector.tensor_tensor(out=st["r"][:], in0=st["lnp"][:],
                                                in1=st["ln2"][:], op=A.subtract)
                    else:
                        st["r"] = st["lnp"]

                def s11():
                    nc.vector.tensor_reduce(out=red[:, cs], in_=st["r"][:],
                                            axis=mybir.AxisListType.X, op=A.add)

                return [s0, s1, s2, s3, s4, s5, s6, s7, s8, s9, s10, s11]

            stage_lists = [make_stages(*s) for s in sides]
            NSTAGE = 12
            for wave in range(len(sides) + NSTAGE - 1):
                for k in range(len(sides)):
                    s = wave - k
                    if 0 <= s < NSTAGE:
                        stage_lists[k][s]()

            # ---- losses, tie-aware argmin, selection ----
            tsum2 = pp_pool.tile([P, NCAND], f32)
            nc.vector.tensor_tensor(out=tsum2[:], in0=posred[:], in1=negred[:],
                                    op=A.add)
            total = pp_pool.tile([P, NCAND], f32)
            nc.vector.tensor_scalar(out=total[:], in0=tsum2[:],
                                    scalar1=invlg[:, :1], scalar2=None, op0=A.mult)
            minv = pp_pool.tile([P, 1], f32)
            nc.vector.tensor_reduce(out=minv[:], in_=total[:],
                                    axis=mybir.AxisListType.X, op=A.min)
            eqm = pp_pool.tile([P, NCAND], i32)
            nc.vector.tensor_scalar(out=eqm[:], in0=total[:], scalar1=minv[:, :1],
                                    scalar2=None, op0=A.is_equal)
            bigc = pp_pool.tile([P, 1], f32)
            nc.vector.memset(bigc[:], BIGF)
            score = pp_pool.tile([P, NCAND], f32)
            nc.vector.select(out=score[:], mask=eqm[:], on_true=pp_rep[:],
                             on_false=bigc[:].to_broadcast([P, NCAND]))
            pstar = pp_pool.tile([P, 1], f32)
            nc.vector.tensor_reduce(out=pstar[:], in_=score[:],
                                    axis=mybir.AxisListType.X, op=A.min)
            mask2 = pp_pool.tile([P, NCAND], f32)
            nc.vector.tensor_scalar(out=mask2[:], in0=score[:], scalar1=pstar[:, :1],
                                    scalar2=None, op0=A.is_equal)

            sel = pp_pool.tile([P, 8], f32)
            nc.vector.memset(sel[:], 0.0)
            scrap = pp_pool.tile([P, NCAND], f32)
            for col, val in ((0, posred), (1, negred), (3, cncf)):
                nc.vector.scalar_tensor_tensor(
                    out=scrap[:], in0=mask2[:], scalar=1.0, in1=val[:],
                    op0=A.mult, op1=A.mult, accum_out=sel[:, col:col + 1])
            nc.sync.dma_start(out=out_sel[:], in_=sel[:])

    _legalize_waits(nc)
    return nc


def _build_launch_B():
    import concourse.bass as bass
    import concourse.mybir as mybir
    from concourse.tile import TileContext

    i32 = mybir.dt.int32
    nc = bass.Bass()
    dp = nc.declare_dram_parameter
    shard_in = dp("shard_in", [SHARD, TP], i32, isOutput=False)
    scat_idx_in = dp("scat_idx", [P, 1], i32, isOutput=False)
    scat_val_in = dp("scat_val", [P, TP], i32, isOutput=False)
    shard_out = dp("shard_out", [SHARD, TP], i32, isOutput=True)

    with TileContext(nc) as tc:
        with tc.tile_pool(name="sbuf", bufs=1) as pool:
            nc.sync.dma_start(out=shard_out[:], in_=shard_in[:])
            sidx = pool.tile([P, 1], i32)
            nc.sync.dma_start(out=sidx[:], in_=scat_idx_in[:])
            sval = pool.tile([P, TP], i32)
            nc.sync.dma_start(out=sval[:], in_=scat_val_in[:])
            nc.gpsimd.indirect_dma_start(
                out=shard_out[:],
                out_offset=bass.IndirectOffsetOnAxis(ap=sidx[:, :1], axis=0),
                in_=sval[:], in_offset=None,
                bounds_check=SHARD - 1, oob_is_err=False)
    _legalize_waits(nc)
    return nc


# --------------------------------------------------------------------------
# host-side driver
# --------------------------------------------------------------------------

def _consts():
    flip_jk = np.repeat((1 << np.arange(H)).astype(np.int32), K)       # [225]
    tsum = np.zeros((P, NB), np.float32)
    tsum[np.arange(P), np.arange(P) // TP] = 1.0
    trep = np.zeros((NB, P), np.float32)
    trep[np.arange(P) // TP, np.arange(P)] = 1.0
    trepq = np.zeros((2 * NB, 2 * P), np.float32)
    trepq[np.arange(P) // TP, np.arange(P)] = 1.0                 # q0 block
    trepq[NB + np.arange(P) // TP, P + np.arange(P)] = 1.0        # q1 block
    tsel = np.zeros((P, LDW), np.int32)
    tsel[np.arange(P), np.arange(P) % TP] = 1
    ident64 = np.eye(DMAX, dtype=np.float32)
    return flip_jk, tsum, trep, trepq, tsel, ident64


def _wrap(idx):
    return np.where(idx < 0, idx + NNODES, idx).astype(np.int32)


def _numpy_ref(lut, sta_ind, locations, degree, pos_ind, neg_ind, random_masks, perm):
    """Exact numpy fallback mirroring reference.py (used only if lut is not
    the bit-length table)."""
    mask = pos_ind != -1
    lg = mask.sum(axis=1).astype(np.float32)
    sta_loc = locations[sta_ind]
    pos_loc = np.where(mask[:, :, None], locations[np.where(mask, pos_ind, 0)], -1)
    neg_loc = np.where(mask[:, :, None], locations[np.where(mask, neg_ind, 0)], -1)
    flip = (1 << np.arange(H, dtype=np.int32))[None, :, None]
    flipped = sta_loc[:, None, :] ^ flip
    cand = (flipped[:, :, None, :] ^ random_masks).reshape(BATCH, H * K, TP)
    cnc_loc = np.concatenate([cand, sta_loc[:, None, :]], axis=1)[:, perm, :]
    m1 = mask[:, :, None]
    m2 = mask[:, :, None, None]

    def dist(xor, m):
        return np.where(m, lut[np.where(m, xor, 0)], -1.0).astype(np.float32)

    dsp = dist(sta_loc[:, None, :] ^ pos_loc, m1)
    dsps = dsp.sum(axis=-1)
    dsn = dist(sta_loc[:, None, :] ^ neg_loc, m1)
    dsns = dsn.sum(axis=-1)
    dpc = dist(cnc_loc[:, None, :, :] ^ pos_loc[:, :, None, :], m2)
    dnc = dist(cnc_loc[:, None, :, :] ^ neg_loc[:, :, None, :], m2)
    dnp_ = np.where(m2, (dpc - dsp[:, :, None, :] + dsps[:, :, None, None]) / TP, 0.0)
    dnn = np.where(m2, (dnc - dsn[:, :, None, :] + dsns[:, :, None, None]) / TP, 1000.0)

    def p_(dis, ig2):
        deg1 = degree[sta_ind][:, None]
        ig2w = np.where(ig2 < 0, ig2 + NNODES, ig2)
        deg2 = degree[ig2w]
        log_ap = np.log(((deg1 + 1) * (deg2 + 1)).astype(np.float32))
        aaq = (dis + EPS) / log_ap[:, :, None, None]
        return 1.0 / (1.0 + aaq ** GAMMA / ALPHA)

    pos_loss = -np.sum(np.log(p_(dnp_, pos_ind)), axis=1) / lg[:, None, None]
    neg_loss = -np.sum(np.log(1.0 + EPS - p_(dnn, neg_ind)), axis=1) / lg[:, None, None]
    total = POS_RATIO * pos_loss + neg_loss
    index = np.argmin(total, axis=1)
    sel_loc = np.take_along_axis(cnc_loc, index[:, None, :], axis=1)[:, 0, :]
    new_loc = locations.copy()
    new_loc[sta_ind] = sel_loc

    def gsel(x):
        return np.take_along_axis(x, index[:, None, :], axis=1)[:, 0, :]

    return (np.stack([gsel(total).mean(), gsel(pos_loss).mean(),
                      gsel(neg_loss).mean()]).astype(np.float32),
            new_loc.astype(np.int32))


def _ensure_ntff_hook():
    """Register the NTFF profiling hook this image's antenv lacks, and stub
    out the artifact upload (no bucket access here)."""
    import sys, types
    if "antenv.axon_hooks" not in sys.modules:
        mod = types.ModuleType("antenv.axon_hooks")
        holder = {}
        mod.set_axon_ntff_profile_hook = lambda h: holder.__setitem__("h", h)
        mod.get_axon_ntff_profile_hook = lambda: holder.get("h")
        try:
            from trn_agent_boot.trn_boot import _ntff_profile_via_ctypes
            hook = _ntff_profile_via_ctypes("/opt/axon/libaxon_pjrt.so")
            if hook is not None:
                holder["h"] = hook
        except Exception:
            pass
        sys.modules["antenv.axon_hooks"] = mod
        import antenv
        antenv.axon_hooks = mod
    import concourse.bass_utils as bu
    bu.upload_artifacts = lambda tmpdir: f"local:{tmpdir}"


def kernel(lut, sta_ind, locations, degree, pos_ind, neg_ind, random_masks, perm):
    lut = np.asarray(lut, np.float32)
    sta_ind = np.asarray(sta_ind, np.int32)
    locations = np.asarray(locations, np.int32)
    degree = np.asarray(degree, np.int32)
    pos_ind = np.asarray(pos_ind, np.int32)
    neg_ind = np.asarray(neg_ind, np.int32)
    random_masks = np.asarray(random_masks, np.int32)
    perm = np.asarray(perm, np.int32)

    tab = _bitlen_table()
    if np.array_equal(lut, tab):
        fix8192 = False
    else:
        d = tab - lut
        only8192 = (d[NLOC // 4] == 1.0) and (np.count_nonzero(d) == 1)
        if only8192:
            fix8192 = True
        else:
            return _numpy_ref(lut, sta_ind, locations, degree, pos_ind, neg_ind,
                              random_masks, perm)

    from concourse.bass_utils import run_bass_kernel_spmd

    key = ("A", fix8192)
    if key not in _CACHE:
        _CACHE[key] = _build_launch_A(fix8192)
        _CACHE["B"] = _build_launch_B()
    _CACHE["A"] = _CACHE[key]

    flip_jk, tsum, trep, trepq, tsel, ident64 = _consts()
    pp = np.empty(NCAND, np.float32)
    pp[perm] = np.arange(NCAND, dtype=np.float32)
    pp_rep = np.tile(pp[None, :], (P, 1))
    flip_rep = np.tile(flip_jk[None, :], (P, 1)).astype(np.int32)

    locdeg = np.zeros((NNODES, LDW), np.int32)
    locdeg[:, 0:TP] = locations
    locdeg[:, 8] = degree

    in_maps_A = []
    for m in range(NCORES):
        bs = slice(m * NB, (m + 1) * NB)
        pos_l, neg_l = pos_ind[bs], neg_ind[bs]
        in_maps_A.append(dict(
            locdeg=locdeg,
            sta_bt=np.repeat(sta_ind[bs], TP).reshape(P, 1).astype(np.int32),
            posT=np.ascontiguousarray(_wrap(pos_l).T),
            negT=np.ascontiguousarray(_wrap(neg_l).T),
            pos_bt=np.repeat(pos_l, TP, axis=0).astype(np.int32),
            rmask_bt=np.ascontiguousarray(
                random_masks[bs].transpose(0, 3, 1, 2).reshape(P, H * K)).astype(np.int32),
            flip_rep=flip_rep, pp_rep=pp_rep, ident64=ident64,
            tsum=tsum, trep=trep, trepq=trepq, tsel=tsel,
        ))
    import os, tempfile
    trace = bool(os.environ.get("KERNEL_TRACE"))
    tkw = {}
    global LAST_TRACE_DIR_A, LAST_TRACE_DIR_B
    if trace:
        _ensure_ntff_hook()
        LAST_TRACE_DIR_A = tempfile.mkdtemp(prefix="ktrA_")
        tkw = dict(trace=True, tmpdir=LAST_TRACE_DIR_A)
    resA = run_bass_kernel_spmd(_CACHE["A"], in_maps_A, list(range(NCORES)), **tkw)
    global LAST_EXEC_NS_A
    LAST_EXEC_NS_A = resA.exec_time_ns
    sel = np.stack([resA.results[m]["out_sel"] for m in range(NCORES)])  # [8,128,8]

    sel_bt = sel.reshape(NCORES, NB, TP, 8).reshape(BATCH, TP, 8)
    lg = (pos_ind != -1).sum(axis=1).astype(np.float32)                  # [128]
    pos_sel = sel_bt[:, :, 0] / lg[:, None]
    neg_sel = sel_bt[:, :, 1] / lg[:, None]
    tl = float((pos_sel + neg_sel).mean())
    pl = float(pos_sel.mean())
    nl = float(neg_sel.mean())
    sel_loc = np.rint(sel_bt[:, :, 3]).astype(np.int32)                  # [128, 8]

    # launch B: shard copy + disjoint scatter
    in_maps_B = []
    shard_of = sta_ind // SHARD
    for m in range(NCORES):
        sidx = np.full((P, 1), 1 << 20, np.int32)
        sval = np.zeros((P, TP), np.int32)
        rows = np.where(shard_of == m)[0]
        sidx[:len(rows), 0] = sta_ind[rows] - m * SHARD
        sval[:len(rows)] = sel_loc[rows]
        in_maps_B.append(dict(
            shard_in=np.ascontiguousarray(locations[m * SHARD:(m + 1) * SHARD]),
            scat_idx=sidx, scat_val=sval,
        ))
    tkwB = {}
    if trace:
        LAST_TRACE_DIR_B = tempfile.mkdtemp(prefix="ktrB_")
        tkwB = dict(trace=True, tmpdir=LAST_TRACE_DIR_B)
    resB = run_bass_kernel_spmd(_CACHE["B"], in_maps_B, list(range(NCORES)), **tkwB)
    global LAST_EXEC_NS_B, LAST_EXEC_NS
    LAST_EXEC_NS_B = resB.exec_time_ns
    if LAST_EXEC_NS_A is not None and LAST_EXEC_NS_B is not None:
        LAST_EXEC_NS = LAST_EXEC_NS_A + LAST_EXEC_NS_B
    new_loc = np.concatenate(
        [resB.results[m]["shard_out"] for m in range(NCORES)], axis=0)

    return (np.stack([tl, pl, nl]).astype(np.float32), new_loc.astype(np.int32))
